# revision 1
# baseline (speedup 1.0000x reference)
"""MAE self-attention (sparse_attention) Trainium2 Bass kernel, v3.

Sharding: 8 cores = batch(2) x head-groups(4 groups of 3 heads).

Structure (see git history for the evolution):
  - The embx key (key 0 of 2049) is handled on the HOST as a rank-1
    correction, so the device sees exactly 2048 keys = 16 aligned tiles
    (no padded tile; the no-attend diagonal lands on block diagonals).
  - kv projection (bf16): k^T via W-stationary matmuls with head-packed
    m-tiles [k_h0|k_h1] and [k_h2|k_h2]; v via xT-stationary matmuls.
  - scores^T[j, q]: row-packed matmul pairs on PE quadrant rows 0-63 /
    64-127.  Heads h0/h1 pair with EACH OTHER (same query chunk, two psum
    halves), so no kT or q duplication is needed; h2 pairs with itself
    via the [k_h2|k_h2] projection layout (only q_h2 ships duplicated).
  - p = exp(scale*scores + keybias): mostly ACT (Exp activation, masked
    keys underflow to exactly 0); a tunable subset of tiles runs on DVE
    via a bf16 Schraudolph exp (one tensor_scalar to int16 bits; masked
    keys get multiplier 0 -> +0.0).  Diagonal zeroed by [128,128] bf16
    mask multiplies on DVE.
  - pv TRANSPOSED: out[q, d] accumulates in PSUM [128q, 4, 65] (four
    128-query chains share one bank) over the 16 key tiles with pt
    stationary; column 64 is the ones-column -> softmax denominator.
  - PE warm-up spin amortizes the tensor engine's DVFS ramp; inputs
    arrive as a few large DMAs ordered by first use (the DMA transfer
    stage is a serial resource).
Host divides by the denominator after adding the embx rank-1 term.
"""

import ml_dtypes
import numpy as np

import concourse.bacc as bacc
import concourse.bass as bass  # noqa: F401
import concourse.mybir as mybir
import concourse.tile as tile
from concourse.bass_utils import run_bass_kernel_spmd

F32 = mybir.dt.float32
BF16 = mybir.dt.bfloat16
I16 = mybir.dt.int16
Exp = mybir.ActivationFunctionType.Exp
AluMult = mybir.AluOpType.mult
AluAdd = mybir.AluOpType.add

B = 2
S = 2048          # queries; also device-side keys (hidden states only)
HID = 768
H = 12
D = 64
G = 3             # heads per core
NCORE = 8
NT = 16           # key tiles of 128
KC = HID // 128   # 6 contraction chunks
NEG = -10000.0
SCALE = 0.125     # D ** -0.5

# Schraudolph bf16 exp: exp(y) ~= bitcast_bf16(int16(y*SA16 + SB16)).
# SA16 = 128/ln2; SB16 tuned numerically (rms rel err ~1.8%, max ~4.3%;
# within 0.25 of optimal for either round or trunc float->int converts).
SA16 = 184.66496414152556
SB16 = 16248.75
# key-tiles per block whose exp runs on DVE instead of ACT
OFFLOAD_T = (1, 4, 6, 9, 11, 14)

WCOLS = 448       # W layout: [k_h0|k_h1 | k_h2|k_h2 | v_h0 v_h1 v_h2 (192)]
LOOKAHEAD = 2


def _build_nc(reps=1):
    nc = bacc.Bacc(None, target_bir_lowering=False)

    # pre-chunked host layouts: partition dim first so each input needs
    # only a few large DMAs (the transfer stage is serial; ~900ns fixed
    # semaphore-propagation cost per transfer)
    xT_d = nc.dram_tensor("xT", [128, KC, S], BF16, kind="ExternalInput")
    w_d = nc.dram_tensor("W", [128, KC, WCOLS], BF16, kind="ExternalInput")
    # qT slot 0 = [q_h0 ; q_h1], slot 1 = [q_h2 ; q_h2]
    qT_d = nc.dram_tensor("qT", [128, 2, S], BF16, kind="ExternalInput")
    # ct = [bk(2) | bv(192) | kb(16) | s1(16) | s2(16)]
    ct_d = nc.dram_tensor("ct", [128, 242], F32, kind="ExternalInput")
    dm_d = nc.dram_tensor("dm", [128, 128], BF16, kind="ExternalInput")
    out_d = nc.dram_tensor("outT", [G, 2, 2, 128, 4 * (D + 1)], F32,
                           kind="ExternalOutput")

    with tile.TileContext(nc) as tc:
        with (
            tc.tile_pool(name="const", bufs=1) as cpool,
            tc.tile_pool(name="pt", bufs=4) as ptpool,
            tc.tile_pool(name="ovec", bufs=4) as opool,
            tc.tile_pool(name="psS", bufs=3, space="PSUM") as pss,
            tc.tile_pool(name="psV", bufs=2, space="PSUM") as psv,
        ):
            xT_sb = cpool.tile([128, KC, S], BF16)
            w_sb = cpool.tile([128, KC, WCOLS], BF16)
            qT_sb = cpool.tile([128, 2, S], BF16)
            kTa_sb = cpool.tile([128, S], BF16)    # [k_h0 ; k_h1]
            kTc_sb = cpool.tile([128, S], BF16)    # [k_h2 ; k_h2]
            v_sb = cpool.tile([128, NT, G, D + 1], BF16)
            ct_sb = cpool.tile([128, 242], F32)
            dm_sb = cpool.tile([128, 128], BF16)
            bk_sb = ct_sb[:, 0:2]
            bv_sb = ct_sb[:, 2:194]
            kb_sb = ct_sb[:, 194:210]
            s1_sb = ct_sb[:, 210:226]
            s2_sb = ct_sb[:, 226:242]

            # PE warm-up: throwaway matmuls so the tensor engine's DVFS
            # ramp (slow p-states for the first ~3us of activity) is spent
            # before the first real projection chain arrives.
            wu_sb = cpool.tile([128, 512], BF16)
            wups = pss.tile([128, 1024], F32, tag="ps", name="wups")
            nc.gpsimd.memset(wu_sb, 0.0)
            for i in range(8):
                nc.tensor.matmul(
                    wups[:, 0:512], wu_sb[:, 0:128], wu_sb,
                    start=True, stop=True,
                )

            # --- input DMAs, all on the SP ring, ordered by first use (the
            # transfer stage is a serial resource, so a second ring buys no
            # bandwidth — and triggers on the ACT ring would hold the ACT
            # sequencer ~700ns each in front of the first exps) ---
            nc.sync.dma_start(out=w_sb[:, :, 0:128], in_=w_d[:, :, 0:128])
            nc.sync.dma_start(out=xT_sb[:, :, 0:256], in_=xT_d[:, :, 0:256])
            nc.sync.dma_start(out=ct_sb, in_=ct_d[:, :])
            nc.sync.dma_start(out=qT_sb[:, 0, :], in_=qT_d[:, 0, :])
            nc.sync.dma_start(out=xT_sb[:, :, 256:512],
                              in_=xT_d[:, :, 256:512])
            nc.sync.dma_start(out=w_sb[:, :, 128:WCOLS],
                              in_=w_d[:, :, 128:WCOLS])
            nc.sync.dma_start(out=xT_sb[:, :, 512:1024],
                              in_=xT_d[:, :, 512:1024])
            nc.sync.dma_start(out=dm_sb, in_=dm_d[:, :])
            nc.sync.dma_start(out=xT_sb[:, :, 1024:1536],
                              in_=xT_d[:, :, 1024:1536])
            nc.sync.dma_start(out=xT_sb[:, :, 1536:2048],
                              in_=xT_d[:, :, 1536:2048])
            nc.sync.dma_start(out=qT_sb[:, 1, :], in_=qT_d[:, 1, :])

            for rep in range(reps):
                # ---- kv projection ----
                def proj_k_chain(ct, c0, w=512):
                    ps = pss.tile([128, 1024], F32, tag="ps")
                    for kc in range(KC):
                        nc.tensor.matmul(
                            ps[:, 0:w],
                            w_sb[:, kc, ct * 128:(ct + 1) * 128],
                            xT_sb[:, kc, c0:c0 + w],
                            start=(kc == 0),
                            stop=(kc == KC - 1),
                        )
                    dst = kTa_sb if ct == 0 else kTc_sb
                    # bias-add on ACT (idle during the projection phase)
                    nc.scalar.add(dst[:, c0:c0 + w], ps[:, 0:w],
                                  bk_sb[:, ct:ct + 1])

                def proj_v_tile(t):
                    ps = pss.tile([128, 1024], F32, tag="ps")
                    for kc in range(KC):
                        nc.tensor.matmul(
                            ps[:, 0:192],
                            xT_sb[:, kc, t * 128:(t + 1) * 128],
                            w_sb[:, kc, 2 * 128:2 * 128 + 192],
                            start=(kc == 0),
                            stop=(kc == KC - 1),
                        )
                    nc.vector.tensor_add(
                        v_sb[:, t, :, 0:D],
                        ps[:, 0:G * D].rearrange("p (h d) -> p h d", h=G),
                        bv_sb.rearrange("p (h d) -> p h d", h=G),
                    )
                    nc.vector.memset(v_sb[:, t, :, D:D + 1], 1.0)

                # ---- attention ----
                # blocks: (pair, qc).  pair 0 = heads h0/h1, query chunk
                # qc*512; pair 1 = h2 self-paired, chunks 2qc / 2qc+1 on the
                # two psum halves.
                blocks = [(0, qc) for qc in range(4)] + [(1, j) for j in (0, 1)]
                pt_tiles = {}

                def emit_scores(pair, qc, t):
                    ps = pss.tile([128, 1024], F32, tag="ps")
                    kT = kTa_sb if pair == 0 else kTc_sb
                    if pair == 0:
                        qA = qT_sb[0:64, 0, qc * 512:(qc + 1) * 512]
                        qB = qT_sb[64:128, 0, qc * 512:(qc + 1) * 512]
                    else:
                        qA = qT_sb[0:64, 1, qc * 1024:qc * 1024 + 512]
                        qB = qT_sb[64:128, 1, qc * 1024 + 512:(qc + 1) * 1024]
                    nc.tensor.matmul(
                        ps[:, 0:512], kT[0:64, t * 128:(t + 1) * 128], qA,
                        start=True, stop=True, tile_position=(0, 0),
                    )
                    nc.tensor.matmul(
                        ps[:, 512:1024], kT[64:128, t * 128:(t + 1) * 128], qB,
                        start=True, stop=True, tile_position=(64, 0),
                    )
                    return ps

                def emit_exp(pair, qc, t, ps, extra=False, bi=None):
                    pt = ptpool.tile([128, 1024], BF16, tag=f"pt{t}")
                    dve = t in OFFLOAD_T or extra
                    # same-engine mask for DVE tiles avoids a Pool hop (two
                    # extra cross-engine semaphore delays) on the pt path
                    meng = nc.vector if dve else nc.gpsimd
                    if dve:
                        nc.vector.tensor_scalar(
                            pt.bitcast(I16), ps,
                            s1_sb[:, t:t + 1], s2_sb[:, t:t + 1],
                            AluMult, AluAdd,
                        )
                    else:
                        nc.scalar.activation(
                            pt, ps, Exp, bias=kb_sb[:, t:t + 1], scale=SCALE
                        )
                    # zero the q == key block diagonal (on the otherwise-idle
                    # GPSIMD engine; pt lives in SBUF which Pool can access)
                    c = (t % 4) * 128
                    if pair == 0:
                        if t // 4 == qc:
                            meng.tensor_mul(
                                pt[:, c:c + 128], pt[:, c:c + 128], dm_sb)
                            meng.tensor_mul(
                                pt[:, 512 + c:512 + c + 128],
                                pt[:, 512 + c:512 + c + 128], dm_sb)
                    else:
                        if t // 4 == 2 * qc:
                            meng.tensor_mul(
                                pt[:, c:c + 128], pt[:, c:c + 128], dm_sb)
                        elif t // 4 == 2 * qc + 1:
                            meng.tensor_mul(
                                pt[:, 512 + c:512 + c + 128],
                                pt[:, 512 + c:512 + c + 128], dm_sb)
                    pt_tiles[(pair, qc, t)] = pt

                # pv chains: chain (h, qt) covers queries qt*128..+128 of
                # head h.  Four consecutive chains of one head share a
                # 1-bank PSUM tile and one output DMA.
                pv_cur = [None]
                ov_cur = [None, None]

                def pt_col(h, qt, t):
                    if h < 2:
                        key = (0, qt // 4, t)
                        col = 512 * h + (qt % 4) * 128
                    else:
                        key = (1, qt // 8, t)
                        col = 512 * ((qt % 8) // 4) + (qt % 4) * 128
                    return pt_tiles[key][:, col:col + 128]

                def emit_pv_chain(h, qt, i0=0, i1=NT, pv=None, tail=False,
                                  order=None):
                    if pv is None:
                        if qt % 4 == 0 and i0 == 0:
                            pv_cur[0] = psv.tile(
                                [128, 4, D + 1], F32, tag="pv",
                                name=f"pv_{rep}_{h}_{qt}")
                        pv = pv_cur[0]
                    for idx in range(i0, i1):
                        t = order[idx] if order else idx
                        nc.tensor.matmul(
                            pv[:, qt % 4, :],
                            pt_col(h, qt, t),
                            v_sb[:, t, h, :],
                            start=(idx == 0),
                            stop=(idx == NT - 1),
                        )
                    if i1 < NT:
                        return pv
                    if tail and qt % 2 == 1:
                        # drain: per-2-chain copies/DMAs so the kernel's last
                        # transfer is small and starts as early as possible
                        if qt % 4 == 1:
                            ov_cur[1] = opool.tile(
                                [128, 4, D + 1], F32, tag="ov",
                                name=f"ovt_{rep}_{h}_{qt}")
                        ov = ov_cur[1]
                        s = (qt % 4) - 1
                        nc.vector.tensor_copy(ov[:, s:s + 2, :],
                                              pv[:, s:s + 2, :])
                        nc.sync.dma_start(
                            out=out_d[h, qt // 8, (qt // 4) % 2, :,
                                      s * (D + 1):(s + 2) * (D + 1)],
                            in_=ov[:, s:s + 2, :].rearrange(
                                "p a b -> p (a b)"),
                        )
                    elif not tail and qt % 4 == 3:
                        ov = opool.tile([128, 4, D + 1], F32, tag="ov",
                                        name=f"ov_{rep}_{h}_{qt}")
                        nc.vector.tensor_copy(ov, pv)
                        nc.sync.dma_start(
                            out=out_d[h, qt // 8, (qt // 4) % 2, :, :],
                            in_=ov.rearrange("p a b -> p (a b)"),
                        )

                # chains of block bi, in emission order (groups of 4)
                def block_chains(bi):
                    pair, qc = blocks[bi]
                    if pair == 0:
                        return ([(0, 4 * qc + i) for i in range(4)]
                                + [(1, 4 * qc + i) for i in range(4)])
                    return [(2, 8 * qc + i) for i in range(8)]

                # Remaining projection work rides inside the attention step
                # stream, timed to the xT column-slice DMA arrivals.
                vq = list(range(NT))
                bwork = {0: {}, 1: {}, 2: {}}
                for st in range(NT):
                    if st == 2:
                        bwork[0][st] = lambda: proj_k_chain(0, 512)
                    elif st == 6:
                        bwork[0][st] = lambda: proj_k_chain(0, 1024)
                    elif st == 10:
                        bwork[0][st] = lambda: proj_k_chain(0, 1536)
                    else:
                        bwork[0][st] = (
                            lambda tt: (lambda: proj_v_tile(tt)))(vq.pop(0))
                for st in (0, 1, 2):
                    bwork[1][st] = (
                        lambda tt: (lambda: proj_v_tile(tt)))(vq.pop(0))
                for i, st in enumerate((0, 2, 4, 6)):
                    bwork[2][st] = (
                        lambda n: (lambda: proj_k_chain(1, 512 * n)))(i)

                # first chain in two 256-wide halves: the leading xT slice
                # is half as large, so the first score steps start earlier
                proj_k_chain(0, 0, 256)
                proj_k_chain(0, 256, 256)

                # flat software pipeline over all (block, t) steps: scores
                # run LOOKAHEAD steps ahead of exp, across block boundaries
                forder = list(range(8, NT)) + list(range(8))
                steps = [(bi, t) for bi in range(len(blocks))
                         for t in range(NT)]
                n_steps = len(steps)
                final = len(blocks) - 1
                prev_ps = {}
                pvqs = {bi: block_chains(bi - 1)
                        for bi in range(1, len(blocks))}
                for i in range(n_steps + LOOKAHEAD):
                    if i < n_steps:
                        bi, pos = steps[i]
                        t = forder[pos] if bi == final else pos
                        pair, qc = blocks[bi]
                        prev_ps[(bi, t)] = emit_scores(pair, qc, t)
                        if pos == 0 and bi >= 2:
                            # leftover pv chains of earlier blocks
                            for b in range(1, bi):
                                while pvqs.get(b):
                                    emit_pv_chain(*pvqs[b].pop(0))
                        if bi in bwork and pos in bwork[bi]:
                            bwork[bi][pos]()
                        elif pos % 2 == 1 and pos >= 3:
                            # pv chains of the previous block on odd steps
                            # (cross-block lookahead exps land first; block
                            # 0's chains also wait for the last v-tile)
                            if pvqs.get(bi):
                                emit_pv_chain(*pvqs[bi].pop(0))
                        if bi == final and pos == 14:
                            while pvqs[final]:
                                emit_pv_chain(*pvqs[final].pop(0))
                        if bi == final and pos == 15:
                            # head chains of the final block's two psum banks
                            # run their first 13 accumulation steps early
                            # (one open accumulation group per bank)
                            fpv = {8: emit_pv_chain(2, 8, 0, 13, order=forder),
                                   12: emit_pv_chain(2, 12, 0, 13,
                                                     order=forder)}
                    j = i - LOOKAHEAD
                    if j >= 0:
                        bj, pj_pos = steps[j]
                        tj = forder[pj_pos] if bj == final else pj_pos
                        pj, qj = blocks[bj]
                        emit_exp(pj, qj, tj, prev_ps.pop((bj, tj)))
                for qt in range(8, 16):
                    if qt in fpv:
                        emit_pv_chain(2, qt, 13, NT, pv=fpv[qt], tail=True,
                                      order=forder)
                    else:
                        emit_pv_chain(2, qt, pv=fpv[qt & ~3], tail=True,
                                      order=forder)

    nc.finalize()
    return nc


_NC = None


def _get_nc():
    global _NC
    if _NC is None:
        _NC = _build_nc()
    return _NC


def _host_prep(hidden_states, embx, expanded_embx, Wkv_w, Wkv_b,
               attention_mask, mlm_mask):
    hs = np.asarray(hidden_states, np.float32)
    qx = np.asarray(expanded_embx, np.float32)
    w = np.asarray(Wkv_w, np.float32)
    bb = np.asarray(Wkv_b, np.float32)
    am = np.asarray(attention_mask).astype(bool)
    mm = np.asarray(mlm_mask).astype(bool)

    valid = (am & ~mm).astype(np.float32)              # (B, S)

    dm = np.ones((128, 128), ml_dtypes.bfloat16)
    np.fill_diagonal(dm, 0.0)

    # per-batch tensors; xT pre-chunked as [128, KC, S]
    xT = [np.ascontiguousarray(
              hs[b].T.astype(ml_dtypes.bfloat16)
              .reshape(KC, 128, S).transpose(1, 0, 2))
          for b in range(B)]
    kbf, s1f, s2f = [], [], []
    for b in range(B):
        v = valid[b]                                   # (S,)
        kb = np.where(v > 0, 0.0, NEG).astype(np.float32)
        s1 = (v * (SA16 * SCALE)).astype(np.float32)
        s2 = (v * SB16).astype(np.float32)
        kbf.append(kb.reshape(NT, 128).T)
        s1f.append(s1.reshape(NT, 128).T)
        s2f.append(s2.reshape(NT, 128).T)

    # per-group weight layouts, pre-chunked as [128, KC, WCOLS]
    wg_l, bk_l, bv_l = [], [], []
    for g in range(4):
        k_cols = slice(192 * g, 192 * g + 192)
        v_cols = slice(768 + 192 * g, 768 + 192 * g + 192)
        wk = w[:, k_cols]                              # (768, 192)
        parts = [wk[:, 0:128],                         # [k_h0 | k_h1]
                 wk[:, 128:192], wk[:, 128:192],       # [k_h2 | k_h2]
                 w[:, v_cols]]                         # v (192)
        wg = np.concatenate(parts, axis=1).astype(ml_dtypes.bfloat16)
        wg_l.append(np.ascontiguousarray(
            wg.reshape(KC, 128, WCOLS).transpose(1, 0, 2)))
        bkk = bb[k_cols]
        bk = np.stack([bkk[0:128],
                       np.concatenate([bkk[128:192], bkk[128:192]])], axis=1)
        bk_l.append(bk.astype(np.float32))
        bv_l.append(np.broadcast_to(
            bb[v_cols], (128, G * D)).astype(np.float32))

    in_maps = []
    for c in range(NCORE):
        b, g = divmod(c, 4)
        ct = np.concatenate(
            [bk_l[g], bv_l[g], kbf[b], s1f[b], s2f[b]], axis=1)
        qg = qx[b][:, 192 * g:192 * g + 192].T         # (192, S)
        qt = np.empty((128, 2, S), ml_dtypes.bfloat16)
        qt[0:64, 0, :] = qg[0:64].astype(ml_dtypes.bfloat16)
        qt[64:128, 0, :] = qg[64:128].astype(ml_dtypes.bfloat16)
        qt[0:64, 1, :] = qg[128:192].astype(ml_dtypes.bfloat16)
        qt[64:128, 1, :] = qt[0:64, 1, :]
        in_maps.append(dict(
            xT=xT[b], W=wg_l[g], qT=np.ascontiguousarray(qt),
            ct=np.ascontiguousarray(ct), dm=dm,
        ))
    return in_maps


def _host_post(results, embx, expanded_embx, Wkv_w, Wkv_b):
    ex = np.asarray(embx, np.float32)                  # (B, 1, HID)
    qx = np.asarray(expanded_embx, np.float32)
    w = np.asarray(Wkv_w, np.float32)
    bb = np.asarray(Wkv_b, np.float32)

    # embx key: k/v projections + per-query weights, on host
    kv_eb = ex[:, 0, :] @ w + bb                       # (B, 2*HID)
    k_eb = kv_eb[:, :HID].reshape(B, H, D)
    v_eb = kv_eb[:, HID:].reshape(B, H, D)
    q3 = qx.reshape(B, S, H, D)
    s_eb = np.einsum("bshd,bhd->bsh", q3, k_eb)        # (B, S, H)
    p_eb = np.exp(SCALE * s_eb.astype(np.float64)).astype(np.float32)

    out = np.empty((B, S, HID), np.float32)
    for c in range(NCORE):
        b, g = divmod(c, 4)
        # [G, 2, 2, 128, 4, 65] -> (h, half, group, slot, row) -> (G, S, 65)
        ot = (results[c]["outT"]
              .reshape(G, 2, 2, 128, 4, D + 1)
              .transpose(0, 1, 2, 4, 3, 5)
              .reshape(G, S, D + 1))
        for h in range(G):
            hh = 3 * g + h
            num = ot[h, :, :D] + p_eb[b, :, hh:hh + 1] * v_eb[b, hh]
            den = ot[h, :, D] + p_eb[b, :, hh]
            out[b, :, 192 * g + 64 * h:192 * g + 64 * h + 64] = (
                num / den[:, None]
            )
    return out


def kernel(hidden_states, embx, expanded_embx, Wkv_w, Wkv_b,
           attention_mask, mlm_mask):
    in_maps = _host_prep(hidden_states, embx, expanded_embx, Wkv_w, Wkv_b,
                         attention_mask, mlm_mask)
    nc = _get_nc()
    res = run_bass_kernel_spmd(nc, in_maps, list(range(NCORE)))
    return _host_post(res.results, embx, expanded_embx, Wkv_w, Wkv_b)



# revision 13
# speedup vs baseline: 1.0107x; 1.0107x over previous
"""MAE self-attention (sparse_attention) Trainium2 Bass kernel, v3.

Sharding: 8 cores = batch(2) x head-groups(4 groups of 3 heads).

Structure (see git history for the evolution):
  - The embx key (key 0 of 2049) is handled on the HOST as a rank-1
    correction, so the device sees exactly 2048 keys = 16 aligned tiles
    (no padded tile; the no-attend diagonal lands on block diagonals).
  - kv projection (bf16): k^T via W-stationary matmuls with head-packed
    m-tiles [k_h0|k_h1] and [k_h2|k_h2]; v via xT-stationary matmuls.
  - scores^T[j, q]: row-packed matmul pairs on PE quadrant rows 0-63 /
    64-127.  Heads h0/h1 pair with EACH OTHER (same query chunk, two psum
    halves), so no kT or q duplication is needed; h2 pairs with itself
    via the [k_h2|k_h2] projection layout (only q_h2 ships duplicated).
  - p = exp(scale*scores + keybias): mostly ACT (Exp activation, masked
    keys underflow to exactly 0); a tunable subset of tiles runs on DVE
    via a bf16 Schraudolph exp (one tensor_scalar to int16 bits; masked
    keys get multiplier 0 -> +0.0).  Diagonal zeroed by [128,128] bf16
    mask multiplies on DVE.
  - pv TRANSPOSED: out[q, d] accumulates in PSUM [128q, 4, 65] (four
    128-query chains share one bank) over the 16 key tiles with pt
    stationary; column 64 is the ones-column -> softmax denominator.
  - PE warm-up spin amortizes the tensor engine's DVFS ramp; inputs
    arrive as a few large DMAs ordered by first use (the DMA transfer
    stage is a serial resource).
Host divides by the denominator after adding the embx rank-1 term.
"""

import ml_dtypes
import numpy as np

import concourse.bacc as bacc
import concourse.bass as bass  # noqa: F401
import concourse.mybir as mybir
import concourse.tile as tile
from concourse.bass_utils import run_bass_kernel_spmd

F32 = mybir.dt.float32
BF16 = mybir.dt.bfloat16
I16 = mybir.dt.int16
Exp = mybir.ActivationFunctionType.Exp
AluMult = mybir.AluOpType.mult
AluAdd = mybir.AluOpType.add

B = 2
S = 2048          # queries; also device-side keys (hidden states only)
HID = 768
H = 12
D = 64
G = 3             # heads per core
NCORE = 8
NT = 16           # key tiles of 128
KC = HID // 128   # 6 contraction chunks
NEG = -10000.0
SCALE = 0.125     # D ** -0.5

# Schraudolph bf16 exp: exp(y) ~= bitcast_bf16(int16(y*SA16 + SB16)).
# SA16 = 128/ln2; SB16 tuned numerically (rms rel err ~1.8%, max ~4.3%;
# within 0.25 of optimal for either round or trunc float->int converts).
SA16 = 184.66496414152556
SB16 = 16248.75
# key-tiles per block whose exp runs on DVE instead of ACT
OFFLOAD_T = (1, 4, 6, 9, 11, 14)

WCOLS = 448       # W layout: [k_h0|k_h1 | k_h2|k_h2 | v_h0 v_h1 v_h2 (192)]
LOOKAHEAD = 2


def _build_nc(reps=1):
    nc = bacc.Bacc(None, target_bir_lowering=False)

    # pre-chunked host layouts: partition dim first so each input needs
    # only a few large DMAs (the transfer stage is serial; ~900ns fixed
    # semaphore-propagation cost per transfer)
    xT_d = nc.dram_tensor("xT", [128, KC, S], BF16, kind="ExternalInput")
    # W flat layout [k01 (6*128) | k22 (6*128) | v (6*192)], kc-major inside
    # each block, so every DMA slice is >=512B-contiguous per partition
    # (contiguous runs below 512B pay a 2x DMA latency multiplier).
    w_d = nc.dram_tensor("W", [128, KC * WCOLS], BF16, kind="ExternalInput")
    # qT slot 0 = [q_h0 ; q_h1], slot 1 = [q_h2 ; q_h2]
    qT_d = nc.dram_tensor("qT", [128, 2, S], BF16, kind="ExternalInput")
    # ct = [bk(2) | bv(192) | kb(16) | s1(16) | s2(16)]
    ct_d = nc.dram_tensor("ct", [128, 242], F32, kind="ExternalInput")
    dm_d = nc.dram_tensor("dm", [128, 128], BF16, kind="ExternalInput")
    out_d = nc.dram_tensor("outT", [G, 2, 2, 128, 4 * (D + 1)], F32,
                           kind="ExternalOutput")

    with tile.TileContext(nc) as tc:
        with (
            tc.tile_pool(name="const", bufs=1) as cpool,
            tc.tile_pool(name="pt", bufs=4) as ptpool,
            tc.tile_pool(name="ovec", bufs=4) as opool,
            tc.tile_pool(name="psS", bufs=3, space="PSUM") as pss,
            tc.tile_pool(name="psV", bufs=2, space="PSUM") as psv,
        ):
            xT_sb = cpool.tile([128, KC, S], BF16)
            w_sb = cpool.tile([128, KC * WCOLS], BF16)
            qT_sb = cpool.tile([128, 2, S], BF16)
            kTa_sb = cpool.tile([128, S], BF16)    # [k_h0 ; k_h1]
            kTc_sb = cpool.tile([128, S], BF16)    # [k_h2 ; k_h2]
            v_sb = cpool.tile([128, NT, G, D + 1], BF16)
            ct_sb = cpool.tile([128, 242], F32)
            dm_sb = cpool.tile([128, 128], BF16)
            bk_sb = ct_sb[:, 0:2]
            bv_sb = ct_sb[:, 2:194]
            kb_sb = ct_sb[:, 194:210]
            s1_sb = ct_sb[:, 210:226]
            s2_sb = ct_sb[:, 226:242]

            # PE warm-up: throwaway matmuls so the tensor engine's DVFS
            # ramp (slow p-states for the first ~3us of activity) is spent
            # before the first real projection chain arrives.
            wu_sb = cpool.tile([128, 512], BF16)
            wups = pss.tile([128, 1024], F32, tag="ps", name="wups")
            nc.gpsimd.memset(wu_sb, 0.0)
            for i in range(8):
                nc.tensor.matmul(
                    wups[:, 0:512], wu_sb[:, 0:128], wu_sb,
                    start=True, stop=True,
                )

            # --- input DMAs.  The transfer stage is one serial resource, so
            # order = first-use order.  The first two ride the ACT ring: its
            # HWDGE pipeline primes in parallel with the SP ring's, so the
            # first transfer starts ~0.5us earlier, and the ACT sequencer is
            # idle until the first bias-add anyway.  ct (the k bias) must land
            # before the first chain's bias-add. ---
            nc.scalar.dma_start(out=w_sb[:, 0:768], in_=w_d[:, 0:768])
            nc.scalar.dma_start(out=ct_sb, in_=ct_d[:, :])
            nc.sync.dma_start(out=xT_sb[:, :, 0:256], in_=xT_d[:, :, 0:256])
            nc.sync.dma_start(out=qT_sb[:, 0, 0:512], in_=qT_d[:, 0, 0:512])
            nc.sync.dma_start(out=xT_sb[:, :, 256:512],
                              in_=xT_d[:, :, 256:512])
            nc.sync.dma_start(out=w_sb[:, 768:KC * WCOLS],
                              in_=w_d[:, 768:KC * WCOLS])
            nc.sync.dma_start(out=dm_sb, in_=dm_d[:, :])
            nc.sync.dma_start(out=xT_sb[:, :, 512:1024],
                              in_=xT_d[:, :, 512:1024])
            nc.sync.dma_start(out=qT_sb[:, 0, 512:2048],
                              in_=qT_d[:, 0, 512:2048])
            nc.sync.dma_start(out=xT_sb[:, :, 1024:1536],
                              in_=xT_d[:, :, 1024:1536])
            nc.sync.dma_start(out=xT_sb[:, :, 1536:2048],
                              in_=xT_d[:, :, 1536:2048])
            nc.sync.dma_start(out=qT_sb[:, 1, :], in_=qT_d[:, 1, :])

            for rep in range(reps):
                # ---- kv projection ----
                def proj_k_chain(ct, c0, w=512, eng=None):
                    ps = pss.tile([128, 1024], F32, tag="ps")
                    for kc in range(KC):
                        nc.tensor.matmul(
                            ps[:, 0:w],
                            w_sb[:, ct * 768 + kc * 128:
                                 ct * 768 + (kc + 1) * 128],
                            xT_sb[:, kc, c0:c0 + w],
                            start=(kc == 0),
                            stop=(kc == KC - 1),
                        )
                    dst = kTa_sb if ct == 0 else kTc_sb
                    # bias-add engine is chosen per-granule to dodge the
                    # in-order queue behind whichever engine is busy with exps
                    if eng is None or eng is nc.scalar:
                        nc.scalar.add(dst[:, c0:c0 + w], ps[:, 0:w],
                                      bk_sb[:, ct:ct + 1])
                    else:
                        eng.tensor_scalar_add(dst[:, c0:c0 + w], ps[:, 0:w],
                                              bk_sb[:, ct:ct + 1])

                def proj_v_tile(t):
                    ps = pss.tile([128, 1024], F32, tag="ps")
                    for kc in range(KC):
                        nc.tensor.matmul(
                            ps[:, 0:192],
                            xT_sb[:, kc, t * 128:(t + 1) * 128],
                            w_sb[:, 1536 + kc * 192:1536 + (kc + 1) * 192],
                            start=(kc == 0),
                            stop=(kc == KC - 1),
                        )
                    nc.vector.tensor_add(
                        v_sb[:, t, :, 0:D],
                        ps[:, 0:G * D].rearrange("p (h d) -> p h d", h=G),
                        bv_sb.rearrange("p (h d) -> p h d", h=G),
                    )
                    nc.vector.memset(v_sb[:, t, :, D:D + 1], 1.0)

                # ---- attention ----
                # blocks: (pair, qc).  pair 0 = heads h0/h1, query chunk
                # qc*512; pair 1 = h2 self-paired, chunks 2qc / 2qc+1 on the
                # two psum halves.
                blocks = [(0, qc) for qc in range(4)] + [(1, j) for j in (0, 1)]
                pt_tiles = {}

                def emit_scores(pair, qc, t):
                    ps = pss.tile([128, 1024], F32, tag="ps")
                    kT = kTa_sb if pair == 0 else kTc_sb
                    if pair == 0:
                        qA = qT_sb[0:64, 0, qc * 512:(qc + 1) * 512]
                        qB = qT_sb[64:128, 0, qc * 512:(qc + 1) * 512]
                    else:
                        qA = qT_sb[0:64, 1, qc * 1024:qc * 1024 + 512]
                        qB = qT_sb[64:128, 1, qc * 1024 + 512:(qc + 1) * 1024]
                    nc.tensor.matmul(
                        ps[:, 0:512], kT[0:64, t * 128:(t + 1) * 128], qA,
                        start=True, stop=True, tile_position=(0, 0),
                    )
                    nc.tensor.matmul(
                        ps[:, 512:1024], kT[64:128, t * 128:(t + 1) * 128], qB,
                        start=True, stop=True, tile_position=(64, 0),
                    )
                    return ps

                def emit_exp(pair, qc, t, ps, extra=False, bi=None):
                    pt = ptpool.tile([128, 1024], BF16, tag=f"pt{t}")
                    dve = t in OFFLOAD_T or extra
                    # same-engine mask for DVE tiles avoids a Pool hop (two
                    # extra cross-engine semaphore delays) on the pt path
                    meng = nc.vector if dve else nc.gpsimd
                    if dve:
                        nc.vector.tensor_scalar(
                            pt.bitcast(I16), ps,
                            s1_sb[:, t:t + 1], s2_sb[:, t:t + 1],
                            AluMult, AluAdd,
                        )
                    else:
                        nc.scalar.activation(
                            pt, ps, Exp, bias=kb_sb[:, t:t + 1], scale=SCALE
                        )
                    # zero the q == key block diagonal (on the otherwise-idle
                    # GPSIMD engine; pt lives in SBUF which Pool can access)
                    c = (t % 4) * 128
                    if pair == 0:
                        if t // 4 == qc:
                            meng.tensor_mul(
                                pt[:, c:c + 128], pt[:, c:c + 128], dm_sb)
                            meng.tensor_mul(
                                pt[:, 512 + c:512 + c + 128],
                                pt[:, 512 + c:512 + c + 128], dm_sb)
                    else:
                        if t // 4 == 2 * qc:
                            meng.tensor_mul(
                                pt[:, c:c + 128], pt[:, c:c + 128], dm_sb)
                        elif t // 4 == 2 * qc + 1:
                            meng.tensor_mul(
                                pt[:, 512 + c:512 + c + 128],
                                pt[:, 512 + c:512 + c + 128], dm_sb)
                    pt_tiles[(pair, qc, t)] = pt

                # pv chains: chain (h, qt) covers queries qt*128..+128 of
                # head h.  Four consecutive chains of one head share a
                # 1-bank PSUM tile and one output DMA.
                pv_cur = [None]
                ov_cur = [None, None]

                def pt_col(h, qt, t):
                    if h < 2:
                        key = (0, qt // 4, t)
                        col = 512 * h + (qt % 4) * 128
                    else:
                        key = (1, qt // 8, t)
                        col = 512 * ((qt % 8) // 4) + (qt % 4) * 128
                    return pt_tiles[key][:, col:col + 128]

                def emit_pv_chain(h, qt, i0=0, i1=NT, pv=None, tail=False,
                                  order=None):
                    if pv is None:
                        if qt % 4 == 0 and i0 == 0:
                            pv_cur[0] = psv.tile(
                                [128, 4, D + 1], F32, tag="pv",
                                name=f"pv_{rep}_{h}_{qt}")
                        pv = pv_cur[0]
                    for idx in range(i0, i1):
                        t = order[idx] if order else idx
                        nc.tensor.matmul(
                            pv[:, qt % 4, :],
                            pt_col(h, qt, t),
                            v_sb[:, t, h, :],
                            start=(idx == 0),
                            stop=(idx == NT - 1),
                        )
                    if i1 < NT:
                        return pv
                    if tail and qt % 2 == 1:
                        # drain: per-2-chain copies/DMAs so the kernel's last
                        # transfer is small and starts as early as possible
                        if qt % 4 == 1:
                            ov_cur[1] = opool.tile(
                                [128, 4, D + 1], F32, tag="ov",
                                name=f"ovt_{rep}_{h}_{qt}")
                        ov = ov_cur[1]
                        s = (qt % 4) - 1
                        nc.vector.tensor_copy(ov[:, s:s + 2, :],
                                              pv[:, s:s + 2, :])
                        nc.sync.dma_start(
                            out=out_d[h, qt // 8, (qt // 4) % 2, :,
                                      s * (D + 1):(s + 2) * (D + 1)],
                            in_=ov[:, s:s + 2, :].rearrange(
                                "p a b -> p (a b)"),
                        )
                    elif not tail and qt % 4 == 3:
                        ov = opool.tile([128, 4, D + 1], F32, tag="ov",
                                        name=f"ov_{rep}_{h}_{qt}")
                        nc.vector.tensor_copy(ov, pv)
                        nc.sync.dma_start(
                            out=out_d[h, qt // 8, (qt // 4) % 2, :, :],
                            in_=ov.rearrange("p a b -> p (a b)"),
                        )

                # chains of block bi, in emission order (groups of 4)
                def block_chains(bi):
                    pair, qc = blocks[bi]
                    if pair == 0:
                        return ([(0, 4 * qc + i) for i in range(4)]
                                + [(1, 4 * qc + i) for i in range(4)])
                    return [(2, 8 * qc + i) for i in range(8)]

                # Remaining projection work rides inside the attention step
                # stream, timed to the xT column-slice DMA arrivals.  bwork
                # values are LISTS of thunks (all emitted after that step's
                # score matmuls, before the next step's).
                def vt(t):
                    return lambda: proj_v_tile(t)

                def kch(ct, c0, w=256, eng=None):
                    return lambda: proj_k_chain(ct, c0, w, eng)

                V, A = nc.vector, nc.scalar
                bwork = {
                    0: {
                        0: [kch(0, 256)],
                        1: [vt(0)],
                        2: [vt(1)],
                        3: [kch(0, 512, eng=V), kch(0, 768, eng=A)],
                        5: [vt(2)],
                        7: [kch(0, 1024, eng=V), kch(0, 1280, eng=A), vt(3)],
                        9: [vt(4), vt(5)],
                        11: [kch(0, 1536, eng=V), kch(0, 1792, eng=A)],
                        13: [vt(6), vt(7)],
                        15: [vt(8), vt(9)],
                    },
                    1: {0: [vt(10), vt(11)], 1: [vt(12), vt(13)],
                        2: [vt(14), vt(15)]},
                    2: {st: [kch(1, 512 * n, eng=V), kch(1, 512 * n + 256,
                                                        eng=A)]
                        for n, st in enumerate((0, 2, 4, 6))},
                }

                # first granules 128 wide: score t0/t1 start earliest
                proj_k_chain(0, 0, 128)
                proj_k_chain(0, 128, 128)

                # flat software pipeline over all (block, t) steps: scores
                # run LOOKAHEAD steps ahead of exp, across block boundaries
                forder = list(range(8, NT)) + list(range(8))
                steps = [(bi, t) for bi in range(len(blocks))
                         for t in range(NT)]
                n_steps = len(steps)
                final = len(blocks) - 1
                prev_ps = {}
                pvqs = {bi: block_chains(bi - 1)
                        for bi in range(1, len(blocks))}
                for i in range(n_steps + LOOKAHEAD):
                    if i < n_steps:
                        bi, pos = steps[i]
                        t = forder[pos] if bi == final else pos
                        pair, qc = blocks[bi]
                        prev_ps[(bi, t)] = emit_scores(pair, qc, t)
                        if pos == 0 and bi >= 2:
                            # leftover pv chains of earlier blocks
                            for b in range(1, bi):
                                while pvqs.get(b):
                                    emit_pv_chain(*pvqs[b].pop(0))
                        if bi in bwork and pos in bwork[bi]:
                            for thunk in bwork[bi][pos]:
                                thunk()
                        elif pos % 2 == 1 and pos >= 3:
                            # pv chains of the previous block on odd steps
                            # (cross-block lookahead exps land first; block
                            # 0's chains also wait for the last v-tile)
                            if pvqs.get(bi):
                                emit_pv_chain(*pvqs[bi].pop(0))
                        if bi == final and pos == 14:
                            while pvqs[final]:
                                emit_pv_chain(*pvqs[final].pop(0))
                        if bi == final and pos == 15:
                            # head chains of the final block's two psum banks
                            # run their first 13 accumulation steps early
                            # (one open accumulation group per bank)
                            fpv = {8: emit_pv_chain(2, 8, 0, 13, order=forder),
                                   12: emit_pv_chain(2, 12, 0, 13,
                                                     order=forder)}
                    j = i - LOOKAHEAD
                    if j >= 0:
                        bj, pj_pos = steps[j]
                        tj = forder[pj_pos] if bj == final else pj_pos
                        pj, qj = blocks[bj]
                        emit_exp(pj, qj, tj, prev_ps.pop((bj, tj)))
                for qt in range(8, 16):
                    if qt in fpv:
                        emit_pv_chain(2, qt, 13, NT, pv=fpv[qt], tail=True,
                                      order=forder)
                    else:
                        emit_pv_chain(2, qt, pv=fpv[qt & ~3], tail=True,
                                      order=forder)

    nc.finalize()
    return nc


_NC = None


def _get_nc():
    global _NC
    if _NC is None:
        _NC = _build_nc()
    return _NC


def _host_prep(hidden_states, embx, expanded_embx, Wkv_w, Wkv_b,
               attention_mask, mlm_mask):
    hs = np.asarray(hidden_states, np.float32)
    qx = np.asarray(expanded_embx, np.float32)
    w = np.asarray(Wkv_w, np.float32)
    bb = np.asarray(Wkv_b, np.float32)
    am = np.asarray(attention_mask).astype(bool)
    mm = np.asarray(mlm_mask).astype(bool)

    valid = (am & ~mm).astype(np.float32)              # (B, S)

    dm = np.ones((128, 128), ml_dtypes.bfloat16)
    np.fill_diagonal(dm, 0.0)

    # per-batch tensors; xT pre-chunked as [128, KC, S]
    xT = [np.ascontiguousarray(
              hs[b].T.astype(ml_dtypes.bfloat16)
              .reshape(KC, 128, S).transpose(1, 0, 2))
          for b in range(B)]
    kbf, s1f, s2f = [], [], []
    for b in range(B):
        v = valid[b]                                   # (S,)
        kb = np.where(v > 0, 0.0, NEG).astype(np.float32)
        s1 = (v * (SA16 * SCALE)).astype(np.float32)
        s2 = (v * SB16).astype(np.float32)
        kbf.append(kb.reshape(NT, 128).T)
        s1f.append(s1.reshape(NT, 128).T)
        s2f.append(s2.reshape(NT, 128).T)

    # per-group weight layouts, flat [128, KC*WCOLS]: blocks
    # [k01 | k22 | v], kc-major inside each block (>=512B contiguous runs)
    wg_l, bk_l, bv_l = [], [], []
    for g in range(4):
        k_cols = slice(192 * g, 192 * g + 192)
        v_cols = slice(768 + 192 * g, 768 + 192 * g + 192)
        wk = w[:, k_cols]                              # (768, 192)
        blocks = [wk[:, 0:128],                        # [k_h0 | k_h1]
                  np.concatenate([wk[:, 128:192], wk[:, 128:192]], axis=1),
                  w[:, v_cols]]                        # v (192)
        flat = np.concatenate(
            [b.reshape(KC, 128, -1).transpose(1, 0, 2).reshape(128, -1)
             for b in blocks], axis=1).astype(ml_dtypes.bfloat16)
        wg_l.append(np.ascontiguousarray(flat))
        bkk = bb[k_cols]
        bk = np.stack([bkk[0:128],
                       np.concatenate([bkk[128:192], bkk[128:192]])], axis=1)
        bk_l.append(bk.astype(np.float32))
        bv_l.append(np.broadcast_to(
            bb[v_cols], (128, G * D)).astype(np.float32))

    in_maps = []
    for c in range(NCORE):
        b, g = divmod(c, 4)
        ct = np.concatenate(
            [bk_l[g], bv_l[g], kbf[b], s1f[b], s2f[b]], axis=1)
        qg = qx[b][:, 192 * g:192 * g + 192].T         # (192, S)
        qt = np.empty((128, 2, S), ml_dtypes.bfloat16)
        qt[0:64, 0, :] = qg[0:64].astype(ml_dtypes.bfloat16)
        qt[64:128, 0, :] = qg[64:128].astype(ml_dtypes.bfloat16)
        qt[0:64, 1, :] = qg[128:192].astype(ml_dtypes.bfloat16)
        qt[64:128, 1, :] = qt[0:64, 1, :]
        in_maps.append(dict(
            xT=xT[b], W=wg_l[g], qT=np.ascontiguousarray(qt),
            ct=np.ascontiguousarray(ct), dm=dm,
        ))
    return in_maps


def _host_post(results, embx, expanded_embx, Wkv_w, Wkv_b):
    ex = np.asarray(embx, np.float32)                  # (B, 1, HID)
    qx = np.asarray(expanded_embx, np.float32)
    w = np.asarray(Wkv_w, np.float32)
    bb = np.asarray(Wkv_b, np.float32)

    # embx key: k/v projections + per-query weights, on host
    kv_eb = ex[:, 0, :] @ w + bb                       # (B, 2*HID)
    k_eb = kv_eb[:, :HID].reshape(B, H, D)
    v_eb = kv_eb[:, HID:].reshape(B, H, D)
    q3 = qx.reshape(B, S, H, D)
    s_eb = np.einsum("bshd,bhd->bsh", q3, k_eb)        # (B, S, H)
    p_eb = np.exp(SCALE * s_eb.astype(np.float64)).astype(np.float32)

    out = np.empty((B, S, HID), np.float32)
    for c in range(NCORE):
        b, g = divmod(c, 4)
        # [G, 2, 2, 128, 4, 65] -> (h, half, group, slot, row) -> (G, S, 65)
        ot = (results[c]["outT"]
              .reshape(G, 2, 2, 128, 4, D + 1)
              .transpose(0, 1, 2, 4, 3, 5)
              .reshape(G, S, D + 1))
        for h in range(G):
            hh = 3 * g + h
            num = ot[h, :, :D] + p_eb[b, :, hh:hh + 1] * v_eb[b, hh]
            den = ot[h, :, D] + p_eb[b, :, hh]
            out[b, :, 192 * g + 64 * h:192 * g + 64 * h + 64] = (
                num / den[:, None]
            )
    return out


def kernel(hidden_states, embx, expanded_embx, Wkv_w, Wkv_b,
           attention_mask, mlm_mask):
    in_maps = _host_prep(hidden_states, embx, expanded_embx, Wkv_w, Wkv_b,
                         attention_mask, mlm_mask)
    nc = _get_nc()
    res = run_bass_kernel_spmd(nc, in_maps, list(range(NCORE)))
    return _host_post(res.results, embx, expanded_embx, Wkv_w, Wkv_b)



# revision 37
# speedup vs baseline: 1.0460x; 1.0349x over previous
"""MAE self-attention (sparse_attention) Trainium2 Bass kernel, v3.

Sharding: 8 cores = batch(2) x head-groups(4 groups of 3 heads).

Structure (see git history for the evolution):
  - The embx key (key 0 of 2049) is handled on the HOST as a rank-1
    correction, so the device sees exactly 2048 keys = 16 aligned tiles
    (no padded tile; the no-attend diagonal lands on block diagonals).
  - kv projection (bf16): k^T via W-stationary matmuls with head-packed
    m-tiles [k_h0|k_h1] and [k_h2|k_h2]; v via xT-stationary matmuls.
  - scores^T[j, q]: row-packed matmul pairs on PE quadrant rows 0-63 /
    64-127.  Heads h0/h1 pair with EACH OTHER (same query chunk, two psum
    halves), so no kT or q duplication is needed; h2 pairs with itself
    via the [k_h2|k_h2] projection layout (only q_h2 ships duplicated).
  - p = exp(scale*scores + keybias): mostly ACT (Exp activation, masked
    keys underflow to exactly 0); a tunable subset of tiles runs on DVE
    via a bf16 Schraudolph exp (one tensor_scalar to int16 bits; masked
    keys get multiplier 0 -> +0.0).  Diagonal zeroed by [128,128] bf16
    mask multiplies on DVE.
  - pv TRANSPOSED: out[q, d] accumulates in PSUM [128q, 4, 65] (four
    128-query chains share one bank) over the 16 key tiles with pt
    stationary; column 64 is the ones-column -> softmax denominator.
  - PE warm-up spin amortizes the tensor engine's DVFS ramp; inputs
    arrive as a few large DMAs ordered by first use (the DMA transfer
    stage is a serial resource).
Host divides by the denominator after adding the embx rank-1 term.
"""

import ml_dtypes
import numpy as np

import concourse.bacc as bacc
import concourse.bass as bass  # noqa: F401
import concourse.mybir as mybir
import concourse.tile as tile
from concourse.bass_utils import run_bass_kernel_spmd

F32 = mybir.dt.float32
BF16 = mybir.dt.bfloat16
I16 = mybir.dt.int16
Exp = mybir.ActivationFunctionType.Exp
AluMult = mybir.AluOpType.mult
AluAdd = mybir.AluOpType.add

B = 2
S = 2048          # queries; also device-side keys (hidden states only)
HID = 768
H = 12
D = 64
G = 3             # heads per core
NCORE = 8
NT = 16           # key tiles of 128
KC = HID // 128   # 6 contraction chunks
NEG = -10000.0
SCALE = 0.125     # D ** -0.5

# Schraudolph bf16 exp: exp(y) ~= bitcast_bf16(int16(y*SA16 + SB16)).
# SA16 = 128/ln2; SB16 tuned numerically (rms rel err ~1.8%, max ~4.3%;
# within 0.25 of optimal for either round or trunc float->int converts).
SA16 = 184.66496414152556
SB16 = 16248.75
# key-tiles per block whose exp runs on DVE instead of ACT
OFFLOAD_T = (1, 4, 6, 9, 11, 14)

WCOLS = 448       # W layout: [k_h0|k_h1 | k_h2|k_h2 | v_h0 v_h1 v_h2 (192)]
LOOKAHEAD = 2


def _build_nc(reps=1):
    nc = bacc.Bacc(None, target_bir_lowering=False)

    # pre-chunked host layouts: partition dim first so each input needs
    # only a few large DMAs (the transfer stage is serial; ~900ns fixed
    # semaphore-propagation cost per transfer)
    xT_d = nc.dram_tensor("xT", [128, KC, S], BF16, kind="ExternalInput")
    # W flat layout [k01 (6*128) | k22 (6*128) | v (6*192)], kc-major inside
    # each block, so every DMA slice is >=512B-contiguous per partition
    # (contiguous runs below 512B pay a 2x DMA latency multiplier).
    w_d = nc.dram_tensor("W", [128, KC * WCOLS], BF16, kind="ExternalInput")
    # qT slot 0 = [q_h0 ; q_h1], slot 1 = [q_h2 ; q_h2]
    qT_d = nc.dram_tensor("qT", [128, 2, S], BF16, kind="ExternalInput")
    # ct = [bk(2) | bv(192) | kb(16) | s1(16) | s2(16)]
    ct_d = nc.dram_tensor("ct", [128, 242], F32, kind="ExternalInput")
    out_d = nc.dram_tensor("outT", [G, 2, 2, 128, 4 * (D + 1)], F32,
                           kind="ExternalOutput")
    # kT / v shipped back so the host can reproduce and subtract the
    # diagonal (q==key) attention term exactly; the device never masks.
    kta_d = nc.dram_tensor("kTao", [128, S], BF16, kind="ExternalOutput")
    ktc_d = nc.dram_tensor("kTco", [64, S], BF16, kind="ExternalOutput")
    v_d = nc.dram_tensor("vo", [128, NT * G * (D + 1)], BF16,
                         kind="ExternalOutput")

    with tile.TileContext(nc) as tc:
        with (
            tc.tile_pool(name="const", bufs=1) as cpool,
            tc.tile_pool(name="pt", bufs=4) as ptpool,
            tc.tile_pool(name="ovec", bufs=4) as opool,
            tc.tile_pool(name="psS", bufs=3, space="PSUM") as pss,
            tc.tile_pool(name="psV", bufs=2, space="PSUM") as psv,
        ):
            xT_sb = cpool.tile([128, KC, S], BF16)
            w_sb = cpool.tile([128, KC * WCOLS], BF16)
            qT_sb = cpool.tile([128, 2, S], BF16)
            kTa_sb = cpool.tile([128, S], BF16)    # [k_h0 ; k_h1]
            kTc_sb = cpool.tile([128, S], BF16)    # [k_h2 ; k_h2]
            v_sb = cpool.tile([128, NT, G, D + 1], BF16)
            ct_sb = cpool.tile([128, 242], F32)
            bk_sb = ct_sb[:, 0:2]
            bv_sb = ct_sb[:, 2:194]
            kb_sb = ct_sb[:, 194:210]
            s1_sb = ct_sb[:, 210:226]
            s2_sb = ct_sb[:, 226:242]

            # PE warm-up: throwaway matmuls so the tensor engine's DVFS
            # ramp (slow p-states for the first ~3us of activity) is spent
            # before the first real projection chain arrives.
            wu_sb = cpool.tile([128, 512], BF16)
            wups = pss.tile([128, 1024], F32, tag="ps", name="wups")
            nc.gpsimd.memset(wu_sb, 0.0)
            for i in range(8):
                nc.tensor.matmul(
                    wups[:, 0:512], wu_sb[:, 0:128], wu_sb,
                    start=True, stop=True,
                )

            # --- input DMAs.  The transfer stage is one serial resource, so
            # order = first-use order.  The first two ride the ACT ring: its
            # HWDGE pipeline primes in parallel with the SP ring's, so the
            # first transfer starts ~0.5us earlier, and the ACT sequencer is
            # idle until the first bias-add anyway.  ct (the k bias) must land
            # before the first chain's bias-add. ---
            nc.scalar.dma_start(out=w_sb[:, 0:768], in_=w_d[:, 0:768])
            nc.scalar.dma_start(out=ct_sb, in_=ct_d[:, :])
            nc.sync.dma_start(out=xT_sb[:, :, 0:256], in_=xT_d[:, :, 0:256])
            nc.sync.dma_start(out=qT_sb[:, 0, 0:512], in_=qT_d[:, 0, 0:512])
            nc.sync.dma_start(out=xT_sb[:, :, 256:512],
                              in_=xT_d[:, :, 256:512])
            nc.sync.dma_start(out=w_sb[:, 768:KC * WCOLS],
                              in_=w_d[:, 768:KC * WCOLS])
            nc.sync.dma_start(out=xT_sb[:, :, 512:1024],
                              in_=xT_d[:, :, 512:1024])
            nc.sync.dma_start(out=xT_sb[:, :, 1024:1536],
                              in_=xT_d[:, :, 1024:1536])
            nc.sync.dma_start(out=xT_sb[:, :, 1536:2048],
                              in_=xT_d[:, :, 1536:2048])
            nc.sync.dma_start(out=qT_sb[:, 0, 512:2048],
                              in_=qT_d[:, 0, 512:2048])
            nc.sync.dma_start(out=qT_sb[:, 1, :], in_=qT_d[:, 1, :])

            for rep in range(reps):
                # ---- kv projection ----
                def proj_k_chain(ct, c0, w=512, eng=None, early=False):
                    # during block 0 the pv banks are idle (no chains yet)
                    # and a 256-col f32 chain fits a [128,4,65] pv slot, so
                    # early projection work keeps out of the score ps ring
                    if early:
                        ps = psv.tile([128, w], F32, tag="pv",
                                      name=f"kf_{ct}_{c0}")
                    else:
                        ps = pss.tile([128, 1024], F32, tag="ps")
                    for kc in range(KC):
                        nc.tensor.matmul(
                            ps[:, 0:w],
                            w_sb[:, ct * 768 + kc * 128:
                                 ct * 768 + (kc + 1) * 128],
                            xT_sb[:, kc, c0:c0 + w],
                            start=(kc == 0),
                            stop=(kc == KC - 1),
                        )
                    dst = kTa_sb if ct == 0 else kTc_sb
                    # bias-add engine is chosen per-granule to dodge the
                    # in-order queue behind whichever engine is busy with exps
                    if eng is None or eng is nc.scalar:
                        nc.scalar.add(dst[:, c0:c0 + w], ps[:, 0:w],
                                      bk_sb[:, ct:ct + 1])
                    else:
                        eng.tensor_scalar_add(dst[:, c0:c0 + w], ps[:, 0:w],
                                              bk_sb[:, ct:ct + 1])

                def proj_v_tile(t, early=False):
                    if early:
                        ps = psv.tile([128, 192], F32, tag="pv",
                                      name=f"vf_{t}")
                    else:
                        ps = pss.tile([128, 1024], F32, tag="ps")
                    for kc in range(KC):
                        nc.tensor.matmul(
                            ps[:, 0:192],
                            xT_sb[:, kc, t * 128:(t + 1) * 128],
                            w_sb[:, 1536 + kc * 192:1536 + (kc + 1) * 192],
                            start=(kc == 0),
                            stop=(kc == KC - 1),
                        )
                    nc.vector.tensor_add(
                        v_sb[:, t, :, 0:D],
                        ps[:, 0:G * D].rearrange("p (h d) -> p h d", h=G),
                        bv_sb.rearrange("p (h d) -> p h d", h=G),
                    )
                    nc.vector.memset(v_sb[:, t, :, D:D + 1], 1.0)

                # ---- attention ----
                # blocks: (pair, qc).  pair 0 = heads h0/h1, query chunk
                # qc*512; pair 1 = h2 self-paired, chunks 2qc / 2qc+1 on the
                # two psum halves.
                blocks = [(0, qc) for qc in range(4)] + [(1, j) for j in (0, 1)]
                pt_tiles = {}

                def emit_scores(pair, qc, t):
                    ps = pss.tile([128, 1024], F32, tag="ps")
                    kT = kTa_sb if pair == 0 else kTc_sb
                    if pair == 0:
                        qA = qT_sb[0:64, 0, qc * 512:(qc + 1) * 512]
                        qB = qT_sb[64:128, 0, qc * 512:(qc + 1) * 512]
                    else:
                        qA = qT_sb[0:64, 1, qc * 1024:qc * 1024 + 512]
                        qB = qT_sb[64:128, 1, qc * 1024 + 512:(qc + 1) * 1024]
                    nc.tensor.matmul(
                        ps[:, 0:512], kT[0:64, t * 128:(t + 1) * 128], qA,
                        start=True, stop=True, tile_position=(0, 0),
                    )
                    nc.tensor.matmul(
                        ps[:, 512:1024], kT[64:128, t * 128:(t + 1) * 128], qB,
                        start=True, stop=True, tile_position=(64, 0),
                    )
                    return ps

                def emit_exp(pair, qc, t, ps, extra=False, bi=None):
                    pt = ptpool.tile([128, 1024], BF16, tag=f"pt{t}")
                    dve = t in OFFLOAD_T or extra
                    if dve:
                        nc.vector.tensor_scalar(
                            pt.bitcast(I16), ps,
                            s1_sb[:, t:t + 1], s2_sb[:, t:t + 1],
                            AluMult, AluAdd,
                        )
                    else:
                        nc.scalar.activation(
                            pt, ps, Exp, bias=kb_sb[:, t:t + 1], scale=SCALE
                        )
                    # the q == key diagonal is NOT masked on device; the
                    # host subtracts its contribution exactly (see kTao/vo)
                    pt_tiles[(pair, qc, t)] = pt

                # pv chains: chain (h, qt) covers queries qt*128..+128 of
                # head h.  Four consecutive chains of one head share a
                # 1-bank PSUM tile and one output DMA.
                pv_cur = [None]
                ov_cur = [None, None]

                def pt_col(h, qt, t):
                    if h < 2:
                        key = (0, qt // 4, t)
                        col = 512 * h + (qt % 4) * 128
                    else:
                        key = (1, qt // 8, t)
                        col = 512 * ((qt % 8) // 4) + (qt % 4) * 128
                    return pt_tiles[key][:, col:col + 128]

                def emit_pv_chain(h, qt, i0=0, i1=NT, pv=None, order=None,
                                  ps_pool=False, drain=True):
                    if pv is None:
                        if ps_pool:
                            # tail chains ride the score-psum ring: each new
                            # request lands on the slot its gating exp (or
                            # predecessor's copy) is about to release
                            pv = pss.tile([128, 4, D + 1], F32, tag="ps",
                                          name=f"tl_{rep}_{qt}")
                        elif qt % 4 == 0 and i0 == 0:
                            pv_cur[0] = psv.tile(
                                [128, 4, D + 1], F32, tag="pv",
                                name=f"pv_{rep}_{h}_{qt}")
                        pv = pv if ps_pool else pv_cur[0]
                    for idx in range(i0, i1):
                        t = order[idx] if order else idx
                        nc.tensor.matmul(
                            pv[:, qt % 4, :],
                            pt_col(h, qt, t),
                            v_sb[:, t, h, :],
                            start=(idx == 0),
                            stop=(idx == NT - 1),
                        )
                    if i1 < NT:
                        return pv
                    if drain and qt % 4 == 3:
                        ov = opool.tile([128, 4, D + 1], F32, tag="ov",
                                        name=f"ov_{rep}_{h}_{qt}")
                        nc.vector.tensor_copy(ov, pv)
                        nc.sync.dma_start(
                            out=out_d[h, qt // 8, (qt // 4) % 2, :, :],
                            in_=ov.rearrange("p a b -> p (a b)"),
                        )
                    return pv

                # chains of block bi, in emission order (groups of 4)
                def block_chains(bi):
                    pair, qc = blocks[bi]
                    if pair == 0:
                        return ([(0, 4 * qc + i) for i in range(4)]
                                + [(1, 4 * qc + i) for i in range(4)])
                    return [(2, 8 * qc + i) for i in range(8)]

                # Remaining projection work rides inside the attention step
                # stream, timed to the xT column-slice DMA arrivals.  bwork
                # values are LISTS of thunks (all emitted after that step's
                # score matmuls, before the next step's).
                def vt(t, early=False):
                    return lambda: proj_v_tile(t, early)

                def kch(ct, c0, w=256, eng=None, early=False):
                    return lambda: proj_k_chain(ct, c0, w, eng, early)

                V, A = nc.vector, nc.scalar
                bwork = {
                    0: {
                        0: [kch(0, 256, early=True)],
                        1: [vt(0, True)],
                        2: [vt(1, True)],
                        3: [kch(0, 512, eng=V, early=True),
                            kch(0, 768, eng=A, early=True)],
                        5: [vt(2, True)],
                        7: [kch(0, 1024, eng=V, early=True),
                            kch(0, 1280, eng=A, early=True), vt(3, True)],
                        9: [kch(0, 1536, eng=V, early=True),
                            kch(0, 1792, eng=A, early=True)],
                        11: [vt(4, True), vt(5, True)],
                        13: [vt(6, True), vt(7, True)],
                        15: [vt(8, True), vt(9, True)],
                    },
                    1: {0: [vt(10), vt(11)], 1: [vt(12), vt(13)],
                        2: [vt(14), vt(15)]},
                    2: {st: [kch(1, 512 * n, eng=V), kch(1, 512 * n + 256,
                                                        eng=A)]
                        for n, st in enumerate((0, 2, 4, 6))},
                }

                # first granules 128 wide: score t0/t1 start earliest
                proj_k_chain(0, 0, 128, early=True)
                proj_k_chain(0, 128, 128, early=True)

                # flat software pipeline over all (block, t) steps: scores
                # run LOOKAHEAD steps ahead of exp, across block boundaries
                forder = list(range(8, NT)) + list(range(8))
                steps = [(bi, t) for bi in range(len(blocks))
                         for t in range(NT)]
                n_steps = len(steps)
                final = len(blocks) - 1
                prev_ps = {}
                pvqs = {bi: block_chains(bi - 1)
                        for bi in range(1, len(blocks))}
                tl = {}
                ovt = [None, None]
                fpv = {}
                def emit_tail(pj_pos):
                    # the remaining 6 chains ride score-psum slots in ring
                    # order: slot(pos13) -> ch9, slot(pos14) -> ch10,
                    # slot(pos15) -> ch11, then ch13/14/15 reuse them as the
                    # copies drain.  PE fills the final-exp latency with
                    # presteps instead of idling.
                    C = lambda qt, lo, hi, **kw: emit_pv_chain(
                        2, qt, lo, hi, order=forder, ps_pool=True,
                        drain=False, **kw)

                    def copy(qt, pv, eng):
                        g = (qt - 8) // 4
                        ov = ovt[g]
                        s = qt % 4
                        eng_map = {0: nc.vector.tensor_copy,
                                   1: nc.scalar.copy}
                        eng_map[eng](ov[:, s:s + 1, :], pv[:, s:s + 1, :])

                    def dma(qt):
                        g = (qt - 8) // 4
                        s = (qt % 4) - 1
                        nc.sync.dma_start(
                            out=out_d[2, 1, g, :,
                                      s * (D + 1):(s + 2) * (D + 1)],
                            in_=ovt[g][:, s:s + 2, :].rearrange(
                                "p a b -> p (a b)"),
                        )

                    if pj_pos == 13:
                        ovt[0] = opool.tile([128, 4, D + 1], F32, tag="ov",
                                            name=f"ovt0_{rep}")
                        ovt[1] = opool.tile([128, 4, D + 1], F32, tag="ov",
                                            name=f"ovt1_{rep}")
                        tl[9] = C(9, 0, 14)
                        fpv[8] = emit_pv_chain(2, 8, 13, 14, pv=fpv[8],
                                               order=forder)
                        fpv[12] = emit_pv_chain(2, 12, 13, 14, pv=fpv[12],
                                                order=forder)
                    elif pj_pos == 14:
                        tl[10] = C(10, 0, 15)
                        tl[9] = C(9, 14, 15, pv=tl[9])
                        fpv[8] = emit_pv_chain(2, 8, 14, 15, pv=fpv[8],
                                               order=forder)
                        fpv[12] = emit_pv_chain(2, 12, 14, 15, pv=fpv[12],
                                                order=forder)
                    else:
                        emit_pv_chain(2, 8, 15, 16, pv=fpv[8], order=forder)
                        C(9, 15, 16, pv=tl[9])
                        copy(8, fpv[8], 0)
                        copy(9, tl[9], 1)
                        dma(9)
                        C(10, 15, 16, pv=tl[10])
                        tl[11] = C(11, 0, 16)
                        copy(10, tl[10], 0)
                        copy(11, tl[11], 1)
                        dma(11)
                        emit_pv_chain(2, 12, 15, 16, pv=fpv[12], order=forder)
                        copy(12, fpv[12], 0)
                        tl[13] = C(13, 0, 16)
                        copy(13, tl[13], 1)
                        dma(13)
                        tl[14] = C(14, 0, 16)
                        copy(14, tl[14], 0)
                        tl[15] = C(15, 0, 16)
                        copy(15, tl[15], 1)
                        dma(15)

                for i in range(n_steps + LOOKAHEAD):
                    if i < n_steps:
                        bi, pos = steps[i]
                        t = forder[pos] if bi == final else pos
                        pair, qc = blocks[bi]
                        prev_ps[(bi, t)] = emit_scores(pair, qc, t)
                        if pos == 0 and bi >= 2:
                            # leftover pv chains of earlier blocks
                            for b in range(1, bi):
                                while pvqs.get(b):
                                    emit_pv_chain(*pvqs[b].pop(0))
                        if bi in bwork and pos in bwork[bi]:
                            for thunk in bwork[bi][pos]:
                                thunk()
                        elif pos % 2 == 1 and pos >= 3:
                            # pv chains of the previous block on odd steps
                            # (cross-block lookahead exps land first; block
                            # 0's chains also wait for the last v-tile)
                            if pvqs.get(bi):
                                emit_pv_chain(*pvqs[bi].pop(0))
                        if bi == final and pos == 14:
                            while pvqs[final]:
                                emit_pv_chain(*pvqs[final].pop(0))
                        if bi == final and pos == 15:
                            # chains 8/12 prestep in the two pv banks
                            # (one open accumulation group per bank)
                            fpv = {8: emit_pv_chain(2, 8, 0, 13, order=forder),
                                   12: emit_pv_chain(2, 12, 0, 13,
                                                     order=forder)}
                    j = i - LOOKAHEAD
                    if j >= 0:
                        bj, pj_pos = steps[j]
                        tj = forder[pj_pos] if bj == final else pj_pos
                        pj, qj = blocks[bj]
                        emit_exp(pj, qj, tj, prev_ps.pop((bj, tj)))
                        if bj == final and pj_pos >= 13:
                            emit_tail(pj_pos)


    nc.finalize()
    return nc


_NC = None


def _get_nc():
    global _NC
    if _NC is None:
        _NC = _build_nc()
    return _NC


def _host_prep(hidden_states, embx, expanded_embx, Wkv_w, Wkv_b,
               attention_mask, mlm_mask):
    hs = np.asarray(hidden_states, np.float32)
    qx = np.asarray(expanded_embx, np.float32)
    w = np.asarray(Wkv_w, np.float32)
    bb = np.asarray(Wkv_b, np.float32)
    am = np.asarray(attention_mask).astype(bool)
    mm = np.asarray(mlm_mask).astype(bool)

    valid = (am & ~mm).astype(np.float32)              # (B, S)

    # per-batch tensors; xT pre-chunked as [128, KC, S]
    xT = [np.ascontiguousarray(
              hs[b].T.astype(ml_dtypes.bfloat16)
              .reshape(KC, 128, S).transpose(1, 0, 2))
          for b in range(B)]
    kbf, s1f, s2f = [], [], []
    for b in range(B):
        v = valid[b]                                   # (S,)
        kb = np.where(v > 0, 0.0, NEG).astype(np.float32)
        s1 = (v * (SA16 * SCALE)).astype(np.float32)
        s2 = (v * SB16).astype(np.float32)
        kbf.append(kb.reshape(NT, 128).T)
        s1f.append(s1.reshape(NT, 128).T)
        s2f.append(s2.reshape(NT, 128).T)

    # per-group weight layouts, flat [128, KC*WCOLS]: blocks
    # [k01 | k22 | v], kc-major inside each block (>=512B contiguous runs)
    wg_l, bk_l, bv_l = [], [], []
    for g in range(4):
        k_cols = slice(192 * g, 192 * g + 192)
        v_cols = slice(768 + 192 * g, 768 + 192 * g + 192)
        wk = w[:, k_cols]                              # (768, 192)
        blocks = [wk[:, 0:128],                        # [k_h0 | k_h1]
                  np.concatenate([wk[:, 128:192], wk[:, 128:192]], axis=1),
                  w[:, v_cols]]                        # v (192)
        flat = np.concatenate(
            [b.reshape(KC, 128, -1).transpose(1, 0, 2).reshape(128, -1)
             for b in blocks], axis=1).astype(ml_dtypes.bfloat16)
        wg_l.append(np.ascontiguousarray(flat))
        bkk = bb[k_cols]
        bk = np.stack([bkk[0:128],
                       np.concatenate([bkk[128:192], bkk[128:192]])], axis=1)
        bk_l.append(bk.astype(np.float32))
        bv_l.append(np.broadcast_to(
            bb[v_cols], (128, G * D)).astype(np.float32))

    in_maps = []
    for c in range(NCORE):
        b, g = divmod(c, 4)
        ct = np.concatenate(
            [bk_l[g], bv_l[g], kbf[b], s1f[b], s2f[b]], axis=1)
        qg = qx[b][:, 192 * g:192 * g + 192].T         # (192, S)
        qt = np.empty((128, 2, S), ml_dtypes.bfloat16)
        qt[0:64, 0, :] = qg[0:64].astype(ml_dtypes.bfloat16)
        qt[64:128, 0, :] = qg[64:128].astype(ml_dtypes.bfloat16)
        qt[0:64, 1, :] = qg[128:192].astype(ml_dtypes.bfloat16)
        qt[64:128, 1, :] = qt[0:64, 1, :]
        in_maps.append(dict(
            xT=xT[b], W=wg_l[g], qT=np.ascontiguousarray(qt),
            ct=np.ascontiguousarray(ct),
        ))
    return in_maps


def _host_post(results, embx, expanded_embx, Wkv_w, Wkv_b,
               attention_mask, mlm_mask):
    ex = np.asarray(embx, np.float32)                  # (B, 1, HID)
    qx = np.asarray(expanded_embx, np.float32)
    w = np.asarray(Wkv_w, np.float32)
    bb = np.asarray(Wkv_b, np.float32)
    am = np.asarray(attention_mask).astype(bool)
    mm = np.asarray(mlm_mask).astype(bool)
    valid = (am & ~mm)                                 # (B, S)
    schr_q = np.zeros(S, bool)                         # per-query exp path
    for t in OFFLOAD_T:
        schr_q[t * 128:(t + 1) * 128] = True

    # embx key: k/v projections + per-query weights, on host
    kv_eb = ex[:, 0, :] @ w + bb                       # (B, 2*HID)
    k_eb = kv_eb[:, :HID].reshape(B, H, D)
    v_eb = kv_eb[:, HID:].reshape(B, H, D)
    q3 = qx.reshape(B, S, H, D)
    s_eb = np.einsum("bshd,bhd->bsh", q3, k_eb)        # (B, S, H)
    p_eb = np.exp(SCALE * s_eb.astype(np.float64)).astype(np.float32)

    out = np.empty((B, S, HID), np.float32)
    for c in range(NCORE):
        b, g = divmod(c, 4)
        # [G, 2, 2, 128, 4, 65] -> (h, half, group, slot, row) -> (G, S, 65)
        ot = (results[c]["outT"]
              .reshape(G, 2, 2, 128, 4, D + 1)
              .transpose(0, 1, 2, 4, 3, 5)
              .reshape(G, S, D + 1))
        # device-side diagonal term: replicate p[q,q] (bf16 score matmul
        # -> exp or Schraudolph -> bf16 pt) and v (bf16), subtract exactly
        kta = np.asarray(results[c]["kTao"])           # (128, S) bf16
        ktc = np.asarray(results[c]["kTco"])           # (64, S) bf16
        vdev = np.asarray(results[c]["vo"]).reshape(
            128, NT, G, D + 1)                         # bf16
        qg = qx[b][:, 192 * g:192 * g + 192].T.astype(
            ml_dtypes.bfloat16).astype(np.float32)     # (192, S)
        kf = np.concatenate(
            [kta.astype(np.float32), ktc.astype(np.float32)])  # (192, S)
        vq = vdev.transpose(1, 0, 2, 3).reshape(S, G, D + 1).astype(
            np.float32)                                # (S, G, 65)
        vld = valid[b]
        for h in range(G):
            hh = 3 * g + h
            sqq = np.einsum("dq,dq->q", qg[64 * h:64 * h + 64],
                            kf[64 * h:64 * h + 64])    # (S,)
            y = sqq * (vld * (SA16 * SCALE)) + vld * SB16
            p_s = np.rint(y).astype(np.int16).view(
                ml_dtypes.bfloat16).astype(np.float32)
            p_e = np.where(
                vld,
                np.exp(SCALE * sqq).astype(
                    ml_dtypes.bfloat16).astype(np.float32),
                0.0)
            p_qq = np.where(schr_q, p_s, p_e)          # (S,)
            num = (ot[h, :, :D] + p_eb[b, :, hh:hh + 1] * v_eb[b, hh]
                   - p_qq[:, None] * vq[:, h, :D])
            den = ot[h, :, D] + p_eb[b, :, hh] - p_qq
            out[b, :, 192 * g + 64 * h:192 * g + 64 * h + 64] = (
                num / den[:, None]
            )
    return out


def kernel(hidden_states, embx, expanded_embx, Wkv_w, Wkv_b,
           attention_mask, mlm_mask):
    in_maps = _host_prep(hidden_states, embx, expanded_embx, Wkv_w, Wkv_b,
                         attention_mask, mlm_mask)
    nc = _get_nc()
    res = run_bass_kernel_spmd(nc, in_maps, list(range(NCORE)))
    return _host_post(res.results, embx, expanded_embx, Wkv_w, Wkv_b,
                      attention_mask, mlm_mask)



# revision 48
# speedup vs baseline: 1.0518x; 1.0056x over previous
"""MAE self-attention (sparse_attention) Trainium2 Bass kernel, v4.

Sharding: 8 cores = batch(2) x head-groups(4 groups of 3 heads).

Structure (v3 -> v4 changes marked *):
  - The embx key (key 0 of 2049) is handled on the HOST as a rank-1
    correction, so the device sees exactly 2048 keys = 16 aligned tiles
    (no padded tile; the no-attend diagonal lands on block diagonals).
  - kv projection (bf16): k^T via W-stationary matmuls with head-packed
    m-tiles [k_h0|k_h1] and [k_h2|k_h2]; v via xT-stationary matmuls.
  - * W ships as one flat [128, KC*WCOLS] block (every DMA slice is
    >=512B-contiguous per partition; smaller runs pay a 2x DMA
    multiplier).  k^T is built from 128/256-col granules so each
    bias-add (rotated over ACT/DVE to dodge the exp queues) only gates
    two score steps, and the granule/v-tile psum rides the pv banks,
    which sit idle until block 1 (the score ps ring never stalls on
    projection WAR).
  - scores^T[j, q]: row-packed matmul pairs on PE quadrant rows 0-63 /
    64-127.  Heads h0/h1 pair with EACH OTHER (same query chunk, two psum
    halves); h2 pairs with itself via the [k_h2|k_h2] layout.
  - p = exp(scale*scores + keybias): mostly ACT (Exp activation, masked
    keys underflow to exactly 0); OFFLOAD_T tiles run on DVE via a bf16
    Schraudolph exp.  Diagonal zeroed by [128,128] bf16 mask multiplies
    on Pool/DVE (host-side diagonal reconstruction was tried and
    reverted: the huge diagonal p ~ e^{|q|^2/8} cannot be replicated
    through bf16 to the accuracy its cancellation needs).
  - pv TRANSPOSED: out[q, d] accumulates in PSUM [128q, 4, 65] over the
    16 key tiles with pt stationary; column 64 is the ones-column ->
    softmax denominator.
  - * tail: the final block's chains 8/12 prestep in the two pv banks;
    chains 9/10/11/13/14/15 ride the score-psum ring in the exact order
    its slots are released by the last exps (slot(pos13)->ch9, ...),
    so PE fills the final-exp latency with chain presteps.  Output
    copies alternate DVE/ACT; one DMA per 4-chain group at the end.
  - PE warm-up spin amortizes the tensor engine's DVFS ramp; inputs
    arrive as a few large DMAs ordered by first use (the DMA transfer
    stage is a serial resource; the first two ride the ACT ring to
    prime the HWDGE pipeline ~0.5us earlier).
Host divides by the denominator after adding the embx rank-1 term.
"""

import ml_dtypes
import numpy as np

import concourse.bacc as bacc
import concourse.bass as bass  # noqa: F401
import concourse.mybir as mybir
import concourse.tile as tile
from concourse.bass_utils import run_bass_kernel_spmd

F32 = mybir.dt.float32
BF16 = mybir.dt.bfloat16
I16 = mybir.dt.int16
Exp = mybir.ActivationFunctionType.Exp
AluMult = mybir.AluOpType.mult
AluAdd = mybir.AluOpType.add

B = 2
S = 2048          # queries; also device-side keys (hidden states only)
HID = 768
H = 12
D = 64
G = 3             # heads per core
NCORE = 8
NT = 16           # key tiles of 128
KC = HID // 128   # 6 contraction chunks
NEG = -10000.0
SCALE = 0.125     # D ** -0.5

# Schraudolph bf16 exp: exp(y) ~= bitcast_bf16(int16(y*SA16 + SB16)).
# SA16 = 128/ln2; SB16 tuned numerically (rms rel err ~1.8%, max ~4.3%;
# within 0.25 of optimal for either round or trunc float->int converts).
SA16 = 184.66496414152556
SB16 = 16248.75
# key-tiles per block whose exp runs on DVE instead of ACT
OFFLOAD_T = (1, 4, 6, 9, 11, 14)

WCOLS = 448       # W layout: [k_h0|k_h1 | k_h2|k_h2 | v_h0 v_h1 v_h2 (192)]
LOOKAHEAD = 2


def _build_nc(reps=1):
    nc = bacc.Bacc(None, target_bir_lowering=False)

    # pre-chunked host layouts: partition dim first so each input needs
    # only a few large DMAs (the transfer stage is serial; ~900ns fixed
    # semaphore-propagation cost per transfer)
    xT_d = nc.dram_tensor("xT", [128, KC, S], BF16, kind="ExternalInput")
    # W flat layout [k01 (6*128) | k22 (6*128) | v (6*192)], kc-major inside
    # each block, so every DMA slice is >=512B-contiguous per partition
    # (contiguous runs below 512B pay a 2x DMA latency multiplier).
    w_d = nc.dram_tensor("W", [128, KC * WCOLS], BF16, kind="ExternalInput")
    # qT slot 0 = [q_h0 ; q_h1], slot 1 = [q_h2 ; q_h2]
    qT_d = nc.dram_tensor("qT", [128, 2, S], BF16, kind="ExternalInput")
    # ct = [bk(2) | bv(192) | kb(16) | s1(16) | s2(16)]
    ct_d = nc.dram_tensor("ct", [128, 242], F32, kind="ExternalInput")
    dm_d = nc.dram_tensor("dm", [128, 128], BF16, kind="ExternalInput")
    out_d = nc.dram_tensor("outT", [G, 2, 2, 128, 4 * (D + 1)], F32,
                           kind="ExternalOutput")

    with tile.TileContext(nc) as tc:
        with (
            tc.tile_pool(name="const", bufs=1) as cpool,
            tc.tile_pool(name="pt", bufs=4) as ptpool,
            tc.tile_pool(name="ovec", bufs=4) as opool,
            tc.tile_pool(name="psS", bufs=3, space="PSUM") as pss,
            tc.tile_pool(name="psV", bufs=2, space="PSUM") as psv,
        ):
            xT_sb = cpool.tile([128, KC, S], BF16)
            w_sb = cpool.tile([128, KC * WCOLS], BF16)
            qT_sb = cpool.tile([128, 2, S], BF16)
            kTa_sb = cpool.tile([128, S], BF16)    # [k_h0 ; k_h1]
            kTc_sb = cpool.tile([128, S], BF16)    # [k_h2 ; k_h2]
            v_sb = cpool.tile([128, NT, G, D + 1], BF16)
            ct_sb = cpool.tile([128, 242], F32)
            dm_sb = cpool.tile([128, 128], BF16)
            bk_sb = ct_sb[:, 0:2]
            bv_sb = ct_sb[:, 2:194]
            kb_sb = ct_sb[:, 194:210]
            s1_sb = ct_sb[:, 210:226]
            s2_sb = ct_sb[:, 226:242]

            # PE warm-up: throwaway matmuls so the tensor engine's DVFS
            # ramp (slow p-states for the first ~3us of activity) is spent
            # before the first real projection chain arrives.
            wu_sb = cpool.tile([128, 512], BF16)
            wups = pss.tile([128, 1024], F32, tag="ps", name="wups")
            nc.gpsimd.memset(wu_sb, 0.0)
            for i in range(8):
                nc.tensor.matmul(
                    wups[:, 0:512], wu_sb[:, 0:128], wu_sb,
                    start=True, stop=True,
                )

            # --- input DMAs.  The transfer stage is one serial resource, so
            # order = first-use order.  The first two ride the ACT ring: its
            # HWDGE pipeline primes in parallel with the SP ring's, so the
            # first transfer starts ~0.5us earlier, and the ACT sequencer is
            # idle until the first bias-add anyway.  ct (the k bias) must land
            # before the first chain's bias-add. ---
            nc.scalar.dma_start(out=w_sb[:, 0:768], in_=w_d[:, 0:768])
            nc.scalar.dma_start(out=ct_sb, in_=ct_d[:, :])
            nc.scalar.dma_start(out=dm_sb, in_=dm_d[:, :])
            nc.sync.dma_start(out=xT_sb[:, :, 0:256], in_=xT_d[:, :, 0:256])
            nc.sync.dma_start(out=xT_sb[:, :, 256:512],
                              in_=xT_d[:, :, 256:512])
            nc.sync.dma_start(out=qT_sb[:, 0, 0:512], in_=qT_d[:, 0, 0:512])
            nc.sync.dma_start(out=w_sb[:, 768:KC * WCOLS],
                              in_=w_d[:, 768:KC * WCOLS])
            nc.sync.dma_start(out=xT_sb[:, :, 512:1024],
                              in_=xT_d[:, :, 512:1024])
            nc.sync.dma_start(out=xT_sb[:, :, 1024:1536],
                              in_=xT_d[:, :, 1024:1536])
            nc.sync.dma_start(out=xT_sb[:, :, 1536:2048],
                              in_=xT_d[:, :, 1536:2048])
            nc.sync.dma_start(out=qT_sb[:, 0, 512:2048],
                              in_=qT_d[:, 0, 512:2048])
            nc.sync.dma_start(out=qT_sb[:, 1, :], in_=qT_d[:, 1, :])

            for rep in range(reps):
                # ---- kv projection ----
                def proj_k_chain(ct, c0, w=512, eng=None, early=False):
                    # during block 0 the pv banks are idle (no chains yet)
                    # and a 256-col f32 chain fits a [128,4,65] pv slot, so
                    # early projection work keeps out of the score ps ring
                    if early:
                        ps = psv.tile([128, w], F32, tag="pv",
                                      name=f"kf_{ct}_{c0}")
                    else:
                        ps = pss.tile([128, 1024], F32, tag="ps")
                    for kc in range(KC):
                        nc.tensor.matmul(
                            ps[:, 0:w],
                            w_sb[:, ct * 768 + kc * 128:
                                 ct * 768 + (kc + 1) * 128],
                            xT_sb[:, kc, c0:c0 + w],
                            start=(kc == 0),
                            stop=(kc == KC - 1),
                        )
                    dst = kTa_sb if ct == 0 else kTc_sb
                    # bias-add engine is chosen per-granule to dodge the
                    # in-order queue behind whichever engine is busy with exps
                    if eng is None or eng is nc.scalar:
                        nc.scalar.add(dst[:, c0:c0 + w], ps[:, 0:w],
                                      bk_sb[:, ct:ct + 1])
                    else:
                        eng.tensor_scalar_add(dst[:, c0:c0 + w], ps[:, 0:w],
                                              bk_sb[:, ct:ct + 1])

                def proj_v_tile(t, early=False):
                    if early:
                        ps = psv.tile([128, 192], F32, tag="pv",
                                      name=f"vf_{t}")
                    else:
                        ps = pss.tile([128, 1024], F32, tag="ps")
                    for kc in range(KC):
                        nc.tensor.matmul(
                            ps[:, 0:192],
                            xT_sb[:, kc, t * 128:(t + 1) * 128],
                            w_sb[:, 1536 + kc * 192:1536 + (kc + 1) * 192],
                            start=(kc == 0),
                            stop=(kc == KC - 1),
                        )
                    nc.vector.tensor_add(
                        v_sb[:, t, :, 0:D],
                        ps[:, 0:G * D].rearrange("p (h d) -> p h d", h=G),
                        bv_sb.rearrange("p (h d) -> p h d", h=G),
                    )
                    nc.vector.memset(v_sb[:, t, :, D:D + 1], 1.0)

                # ---- attention ----
                # blocks: (pair, qc).  pair 0 = heads h0/h1, query chunk
                # qc*512; pair 1 = h2 self-paired, chunks 2qc / 2qc+1 on the
                # two psum halves.
                blocks = [(0, qc) for qc in range(4)] + [(1, j) for j in (0, 1)]
                pt_tiles = {}

                def emit_scores(pair, qc, t):
                    ps = pss.tile([128, 1024], F32, tag="ps")
                    kT = kTa_sb if pair == 0 else kTc_sb
                    if pair == 0:
                        qA = qT_sb[0:64, 0, qc * 512:(qc + 1) * 512]
                        qB = qT_sb[64:128, 0, qc * 512:(qc + 1) * 512]
                    else:
                        qA = qT_sb[0:64, 1, qc * 1024:qc * 1024 + 512]
                        qB = qT_sb[64:128, 1, qc * 1024 + 512:(qc + 1) * 1024]
                    nc.tensor.matmul(
                        ps[:, 0:512], kT[0:64, t * 128:(t + 1) * 128], qA,
                        start=True, stop=True, tile_position=(0, 0),
                    )
                    nc.tensor.matmul(
                        ps[:, 512:1024], kT[64:128, t * 128:(t + 1) * 128], qB,
                        start=True, stop=True, tile_position=(64, 0),
                    )
                    return ps

                def emit_exp(pair, qc, t, ps, extra=False, bi=None):
                    pt = ptpool.tile([128, 1024], BF16, tag=f"pt{t}")
                    dve = t in OFFLOAD_T or extra
                    # same-engine mask for DVE tiles avoids a Pool hop (two
                    # extra cross-engine semaphore delays) on the pt path
                    meng = nc.vector if dve else nc.gpsimd
                    if dve:
                        nc.vector.tensor_scalar(
                            pt.bitcast(I16), ps,
                            s1_sb[:, t:t + 1], s2_sb[:, t:t + 1],
                            AluMult, AluAdd,
                        )
                    else:
                        nc.scalar.activation(
                            pt, ps, Exp, bias=kb_sb[:, t:t + 1], scale=SCALE
                        )
                    # zero the q == key block diagonal (on the otherwise-idle
                    # GPSIMD engine; pt lives in SBUF which Pool can access)
                    c = (t % 4) * 128
                    if pair == 0:
                        if t // 4 == qc:
                            meng.tensor_mul(
                                pt[:, c:c + 128], pt[:, c:c + 128], dm_sb)
                            meng.tensor_mul(
                                pt[:, 512 + c:512 + c + 128],
                                pt[:, 512 + c:512 + c + 128], dm_sb)
                    else:
                        if t // 4 == 2 * qc:
                            meng.tensor_mul(
                                pt[:, c:c + 128], pt[:, c:c + 128], dm_sb)
                        elif t // 4 == 2 * qc + 1:
                            meng.tensor_mul(
                                pt[:, 512 + c:512 + c + 128],
                                pt[:, 512 + c:512 + c + 128], dm_sb)
                    pt_tiles[(pair, qc, t)] = pt

                # pv chains: chain (h, qt) covers queries qt*128..+128 of
                # head h.  Four consecutive chains of one head share a
                # 1-bank PSUM tile and one output DMA.
                pv_cur = [None]
                ov_cur = [None, None]

                def pt_col(h, qt, t):
                    if h < 2:
                        key = (0, qt // 4, t)
                        col = 512 * h + (qt % 4) * 128
                    else:
                        key = (1, qt // 8, t)
                        col = 512 * ((qt % 8) // 4) + (qt % 4) * 128
                    return pt_tiles[key][:, col:col + 128]

                def emit_pv_chain(h, qt, i0=0, i1=NT, pv=None, order=None,
                                  ps_pool=False, drain=True):
                    if pv is None:
                        if ps_pool:
                            # tail chains ride the score-psum ring: each new
                            # request lands on the slot its gating exp (or
                            # predecessor's copy) is about to release
                            pv = pss.tile([128, 4, D + 1], F32, tag="ps",
                                          name=f"tl_{rep}_{qt}")
                        elif qt % 4 == 0 and i0 == 0:
                            pv_cur[0] = psv.tile(
                                [128, 4, D + 1], F32, tag="pv",
                                name=f"pv_{rep}_{h}_{qt}")
                        pv = pv if ps_pool else pv_cur[0]
                    for idx in range(i0, i1):
                        t = order[idx] if order else idx
                        nc.tensor.matmul(
                            pv[:, qt % 4, :],
                            pt_col(h, qt, t),
                            v_sb[:, t, h, :],
                            start=(idx == 0),
                            stop=(idx == NT - 1),
                        )
                    if i1 < NT:
                        return pv
                    if drain and qt % 4 == 3:
                        ov = opool.tile([128, 4, D + 1], F32, tag="ov",
                                        name=f"ov_{rep}_{h}_{qt}")
                        nc.vector.tensor_copy(ov, pv)
                        nc.sync.dma_start(
                            out=out_d[h, qt // 8, (qt // 4) % 2, :, :],
                            in_=ov.rearrange("p a b -> p (a b)"),
                        )
                    return pv

                # chains of block bi, in emission order (groups of 4)
                def block_chains(bi):
                    pair, qc = blocks[bi]
                    if pair == 0:
                        return ([(0, 4 * qc + i) for i in range(4)]
                                + [(1, 4 * qc + i) for i in range(4)])
                    return [(2, 8 * qc + i) for i in range(8)]

                # Remaining projection work rides inside the attention step
                # stream, timed to the xT column-slice DMA arrivals.  bwork
                # values are LISTS of thunks (all emitted after that step's
                # score matmuls, before the next step's).
                def vt(t, early=False):
                    return lambda: proj_v_tile(t, early)

                def kch(ct, c0, w=256, eng=None, early=False):
                    return lambda: proj_k_chain(ct, c0, w, eng, early)

                V, A = nc.vector, nc.scalar
                bwork = {
                    0: {
                        0: [kch(0, 256, early=True)],
                        1: [vt(0, True)],
                        2: [vt(1, True), kch(0, 512, eng=V, early=True)],
                        3: [kch(0, 768, eng=A, early=True)],
                        5: [vt(2, True)],
                        7: [kch(0, 1024, eng=V, early=True),
                            kch(0, 1280, eng=A, early=True), vt(3, True)],
                        9: [kch(0, 1536, eng=V, early=True),
                            kch(0, 1792, eng=A, early=True)],
                        11: [vt(4, True), vt(5, True)],
                        13: [vt(6, True), vt(7, True)],
                        15: [vt(8, True), vt(9, True)],
                    },
                    1: {0: [vt(10), vt(11)], 1: [vt(12), vt(13)],
                        2: [vt(14), vt(15)]},
                    2: {st: [kch(1, 512 * n, eng=V), kch(1, 512 * n + 256,
                                                        eng=A)]
                        for n, st in enumerate((0, 2, 4, 6))},
                }

                # first granules 128 wide: score t0/t1 start earliest
                proj_k_chain(0, 0, 128, early=True)
                proj_k_chain(0, 128, 128, early=True)

                # flat software pipeline over all (block, t) steps: scores
                # run LOOKAHEAD steps ahead of exp, across block boundaries
                forder = list(range(8, NT)) + list(range(8))
                steps = [(bi, t) for bi in range(len(blocks))
                         for t in range(NT)]
                n_steps = len(steps)
                final = len(blocks) - 1
                prev_ps = {}
                pvqs = {bi: block_chains(bi - 1)
                        for bi in range(1, len(blocks))}
                tl = {}
                ovt = [None, None]
                fpv = {}
                def emit_tail(pj_pos):
                    # the remaining 6 chains ride score-psum slots in ring
                    # order: slot(pos13) -> ch9, slot(pos14) -> ch10,
                    # slot(pos15) -> ch11, then ch13/14/15 reuse them as the
                    # copies drain.  PE fills the final-exp latency with
                    # presteps instead of idling.
                    C = lambda qt, lo, hi, **kw: emit_pv_chain(
                        2, qt, lo, hi, order=forder, ps_pool=True,
                        drain=False, **kw)

                    def copy(qt, pv, eng):
                        g = (qt - 8) // 4
                        ov = ovt[g]
                        s = qt % 4
                        eng_map = {0: nc.vector.tensor_copy,
                                   1: nc.scalar.copy}
                        eng_map[eng](ov[:, s:s + 1, :], pv[:, s:s + 1, :])

                    def dma(qt):
                        # group 0 ships whole; group 1 ships as two pairs so
                        # the kernel's very last transfer is small
                        g = (qt - 8) // 4
                        if qt == 11:
                            nc.sync.dma_start(
                                out=out_d[2, 1, 0, :, :],
                                in_=ovt[0].rearrange("p a b -> p (a b)"),
                            )
                        elif qt in (13, 15):
                            s = (qt % 4) - 1
                            nc.sync.dma_start(
                                out=out_d[2, 1, 1, :,
                                          s * (D + 1):(s + 2) * (D + 1)],
                                in_=ovt[1][:, s:s + 2, :].rearrange(
                                    "p a b -> p (a b)"),
                            )

                    if pj_pos == 13:
                        ovt[0] = opool.tile([128, 4, D + 1], F32, tag="ov",
                                            name=f"ovt0_{rep}")
                        ovt[1] = opool.tile([128, 4, D + 1], F32, tag="ov",
                                            name=f"ovt1_{rep}")
                        tl[9] = C(9, 0, 14)
                        fpv[8] = emit_pv_chain(2, 8, 13, 14, pv=fpv[8],
                                               order=forder)
                        fpv[12] = emit_pv_chain(2, 12, 13, 14, pv=fpv[12],
                                                order=forder)
                    elif pj_pos == 14:
                        tl[10] = C(10, 0, 15)
                        tl[9] = C(9, 14, 15, pv=tl[9])
                        fpv[8] = emit_pv_chain(2, 8, 14, 15, pv=fpv[8],
                                               order=forder)
                        fpv[12] = emit_pv_chain(2, 12, 14, 15, pv=fpv[12],
                                                order=forder)
                    else:
                        emit_pv_chain(2, 8, 15, 16, pv=fpv[8], order=forder)
                        C(9, 15, 16, pv=tl[9])
                        copy(8, fpv[8], 0)
                        copy(9, tl[9], 1)
                        dma(9)
                        C(10, 15, 16, pv=tl[10])
                        tl[11] = C(11, 0, 16)
                        copy(10, tl[10], 0)
                        copy(11, tl[11], 1)
                        dma(11)
                        emit_pv_chain(2, 12, 15, 16, pv=fpv[12], order=forder)
                        copy(12, fpv[12], 0)
                        tl[13] = C(13, 0, 16)
                        copy(13, tl[13], 1)
                        dma(13)
                        tl[14] = C(14, 0, 16)
                        copy(14, tl[14], 1)
                        tl[15] = C(15, 0, 16)
                        copy(15, tl[15], 0)
                        dma(15)

                for i in range(n_steps + LOOKAHEAD):
                    if i < n_steps:
                        bi, pos = steps[i]
                        t = forder[pos] if bi == final else pos
                        pair, qc = blocks[bi]
                        prev_ps[(bi, t)] = emit_scores(pair, qc, t)
                        if pos == 0 and bi >= 2:
                            # leftover pv chains of earlier blocks
                            for b in range(1, bi):
                                while pvqs.get(b):
                                    emit_pv_chain(*pvqs[b].pop(0))
                        if bi in bwork and pos in bwork[bi]:
                            for thunk in bwork[bi][pos]:
                                thunk()
                        elif pos % 2 == 1 and pos >= 3:
                            # pv chains of the previous block on odd steps
                            # (cross-block lookahead exps land first; block
                            # 0's chains also wait for the last v-tile)
                            if pvqs.get(bi):
                                emit_pv_chain(*pvqs[bi].pop(0))
                        if bi == final and pos == 14:
                            while pvqs[final]:
                                emit_pv_chain(*pvqs[final].pop(0))
                        if bi == final and pos == 15:
                            # chains 8/12 prestep in the two pv banks
                            # (one open accumulation group per bank)
                            fpv = {8: emit_pv_chain(2, 8, 0, 13, order=forder),
                                   12: emit_pv_chain(2, 12, 0, 13,
                                                     order=forder)}
                    j = i - LOOKAHEAD
                    if j >= 0:
                        bj, pj_pos = steps[j]
                        tj = forder[pj_pos] if bj == final else pj_pos
                        pj, qj = blocks[bj]
                        emit_exp(pj, qj, tj, prev_ps.pop((bj, tj)))
                        if bj == final and pj_pos >= 13:
                            emit_tail(pj_pos)


    nc.finalize()
    return nc


_NC = None


def _get_nc():
    global _NC
    if _NC is None:
        _NC = _build_nc()
    return _NC


def _host_prep(hidden_states, embx, expanded_embx, Wkv_w, Wkv_b,
               attention_mask, mlm_mask):
    hs = np.asarray(hidden_states, np.float32)
    qx = np.asarray(expanded_embx, np.float32)
    w = np.asarray(Wkv_w, np.float32)
    bb = np.asarray(Wkv_b, np.float32)
    am = np.asarray(attention_mask).astype(bool)
    mm = np.asarray(mlm_mask).astype(bool)

    valid = (am & ~mm).astype(np.float32)              # (B, S)

    dm = np.ones((128, 128), ml_dtypes.bfloat16)
    np.fill_diagonal(dm, 0.0)

    # per-batch tensors; xT pre-chunked as [128, KC, S]
    xT = [np.ascontiguousarray(
              hs[b].T.astype(ml_dtypes.bfloat16)
              .reshape(KC, 128, S).transpose(1, 0, 2))
          for b in range(B)]
    kbf, s1f, s2f = [], [], []
    for b in range(B):
        v = valid[b]                                   # (S,)
        kb = np.where(v > 0, 0.0, NEG).astype(np.float32)
        s1 = (v * (SA16 * SCALE)).astype(np.float32)
        s2 = (v * SB16).astype(np.float32)
        kbf.append(kb.reshape(NT, 128).T)
        s1f.append(s1.reshape(NT, 128).T)
        s2f.append(s2.reshape(NT, 128).T)

    # per-group weight layouts, flat [128, KC*WCOLS]: blocks
    # [k01 | k22 | v], kc-major inside each block (>=512B contiguous runs)
    wg_l, bk_l, bv_l = [], [], []
    for g in range(4):
        k_cols = slice(192 * g, 192 * g + 192)
        v_cols = slice(768 + 192 * g, 768 + 192 * g + 192)
        wk = w[:, k_cols]                              # (768, 192)
        blocks = [wk[:, 0:128],                        # [k_h0 | k_h1]
                  np.concatenate([wk[:, 128:192], wk[:, 128:192]], axis=1),
                  w[:, v_cols]]                        # v (192)
        flat = np.concatenate(
            [b.reshape(KC, 128, -1).transpose(1, 0, 2).reshape(128, -1)
             for b in blocks], axis=1).astype(ml_dtypes.bfloat16)
        wg_l.append(np.ascontiguousarray(flat))
        bkk = bb[k_cols]
        bk = np.stack([bkk[0:128],
                       np.concatenate([bkk[128:192], bkk[128:192]])], axis=1)
        bk_l.append(bk.astype(np.float32))
        bv_l.append(np.broadcast_to(
            bb[v_cols], (128, G * D)).astype(np.float32))

    in_maps = []
    for c in range(NCORE):
        b, g = divmod(c, 4)
        ct = np.concatenate(
            [bk_l[g], bv_l[g], kbf[b], s1f[b], s2f[b]], axis=1)
        qg = qx[b][:, 192 * g:192 * g + 192].T         # (192, S)
        qt = np.empty((128, 2, S), ml_dtypes.bfloat16)
        qt[0:64, 0, :] = qg[0:64].astype(ml_dtypes.bfloat16)
        qt[64:128, 0, :] = qg[64:128].astype(ml_dtypes.bfloat16)
        qt[0:64, 1, :] = qg[128:192].astype(ml_dtypes.bfloat16)
        qt[64:128, 1, :] = qt[0:64, 1, :]
        in_maps.append(dict(
            xT=xT[b], W=wg_l[g], qT=np.ascontiguousarray(qt),
            ct=np.ascontiguousarray(ct), dm=dm,
        ))
    return in_maps


def _host_post(results, embx, expanded_embx, Wkv_w, Wkv_b):
    ex = np.asarray(embx, np.float32)                  # (B, 1, HID)
    qx = np.asarray(expanded_embx, np.float32)
    w = np.asarray(Wkv_w, np.float32)
    bb = np.asarray(Wkv_b, np.float32)

    # embx key: k/v projections + per-query weights, on host
    kv_eb = ex[:, 0, :] @ w + bb                       # (B, 2*HID)
    k_eb = kv_eb[:, :HID].reshape(B, H, D)
    v_eb = kv_eb[:, HID:].reshape(B, H, D)
    q3 = qx.reshape(B, S, H, D)
    s_eb = np.einsum("bshd,bhd->bsh", q3, k_eb)        # (B, S, H)
    p_eb = np.exp(SCALE * s_eb.astype(np.float64)).astype(np.float32)

    out = np.empty((B, S, HID), np.float32)
    for c in range(NCORE):
        b, g = divmod(c, 4)
        # [G, 2, 2, 128, 4, 65] -> (h, half, group, slot, row) -> (G, S, 65)
        ot = (results[c]["outT"]
              .reshape(G, 2, 2, 128, 4, D + 1)
              .transpose(0, 1, 2, 4, 3, 5)
              .reshape(G, S, D + 1))
        for h in range(G):
            hh = 3 * g + h
            num = ot[h, :, :D] + p_eb[b, :, hh:hh + 1] * v_eb[b, hh]
            den = ot[h, :, D] + p_eb[b, :, hh]
            out[b, :, 192 * g + 64 * h:192 * g + 64 * h + 64] = (
                num / den[:, None]
            )
    return out


def kernel(hidden_states, embx, expanded_embx, Wkv_w, Wkv_b,
           attention_mask, mlm_mask):
    in_maps = _host_prep(hidden_states, embx, expanded_embx, Wkv_w, Wkv_b,
                         attention_mask, mlm_mask)
    nc = _get_nc()
    res = run_bass_kernel_spmd(nc, in_maps, list(range(NCORE)))
    return _host_post(res.results, embx, expanded_embx, Wkv_w, Wkv_b)



# revision 49
# speedup vs baseline: 1.0530x; 1.0011x over previous
"""MAE self-attention (sparse_attention) Trainium2 Bass kernel, v4.

Sharding: 8 cores = batch(2) x head-groups(4 groups of 3 heads).

Structure (v3 -> v4 changes marked *):
  - The embx key (key 0 of 2049) is handled on the HOST as a rank-1
    correction, so the device sees exactly 2048 keys = 16 aligned tiles
    (no padded tile; the no-attend diagonal lands on block diagonals).
  - kv projection (bf16): k^T via W-stationary matmuls with head-packed
    m-tiles [k_h0|k_h1] and [k_h2|k_h2]; v via xT-stationary matmuls.
  - * W ships as one flat [128, KC*WCOLS] block (every DMA slice is
    >=512B-contiguous per partition; smaller runs pay a 2x DMA
    multiplier).  k^T is built from 128/256-col granules so each
    bias-add (rotated over ACT/DVE to dodge the exp queues) only gates
    two score steps, and the granule/v-tile psum rides the pv banks,
    which sit idle until block 1 (the score ps ring never stalls on
    projection WAR).
  - scores^T[j, q]: row-packed matmul pairs on PE quadrant rows 0-63 /
    64-127.  Heads h0/h1 pair with EACH OTHER (same query chunk, two psum
    halves); h2 pairs with itself via the [k_h2|k_h2] layout.
  - p = exp(scale*scores + keybias): mostly ACT (Exp activation, masked
    keys underflow to exactly 0); OFFLOAD_T tiles run on DVE via a bf16
    Schraudolph exp.  Diagonal zeroed by [128,128] bf16 mask multiplies
    on Pool/DVE (host-side diagonal reconstruction was tried and
    reverted: the huge diagonal p ~ e^{|q|^2/8} cannot be replicated
    through bf16 to the accuracy its cancellation needs).
  - pv TRANSPOSED: out[q, d] accumulates in PSUM [128q, 4, 65] over the
    16 key tiles with pt stationary; column 64 is the ones-column ->
    softmax denominator.
  - * tail: the final block's chains 8/12 prestep in the two pv banks;
    chains 9/10/11/13/14/15 ride the score-psum ring in the exact order
    its slots are released by the last exps (slot(pos13)->ch9, ...),
    so PE fills the final-exp latency with chain presteps.  Output
    copies alternate DVE/ACT; one DMA per 4-chain group at the end.
  - PE warm-up spin amortizes the tensor engine's DVFS ramp; inputs
    arrive as a few large DMAs ordered by first use (the DMA transfer
    stage is a serial resource; the first two ride the ACT ring to
    prime the HWDGE pipeline ~0.5us earlier).
Host divides by the denominator after adding the embx rank-1 term.
"""

import ml_dtypes
import numpy as np

import concourse.bacc as bacc
import concourse.bass as bass  # noqa: F401
import concourse.mybir as mybir
import concourse.tile as tile
from concourse.bass_utils import run_bass_kernel_spmd

F32 = mybir.dt.float32
BF16 = mybir.dt.bfloat16
I16 = mybir.dt.int16
Exp = mybir.ActivationFunctionType.Exp
AluMult = mybir.AluOpType.mult
AluAdd = mybir.AluOpType.add

B = 2
S = 2048          # queries; also device-side keys (hidden states only)
HID = 768
H = 12
D = 64
G = 3             # heads per core
NCORE = 8
NT = 16           # key tiles of 128
KC = HID // 128   # 6 contraction chunks
NEG = -10000.0
SCALE = 0.125     # D ** -0.5

# Schraudolph bf16 exp: exp(y) ~= bitcast_bf16(int16(y*SA16 + SB16)).
# SA16 = 128/ln2; SB16 tuned numerically (rms rel err ~1.8%, max ~4.3%;
# within 0.25 of optimal for either round or trunc float->int converts).
SA16 = 184.66496414152556
SB16 = 16248.75
# key-tiles per block whose exp runs on DVE instead of ACT
OFFLOAD_T = (1, 4, 6, 9, 11, 14)

WCOLS = 384       # W layout: [k_h0|k_h1 (128) | k_h2 (64) | v (192)]
LOOKAHEAD = 2


def _build_nc(reps=1):
    nc = bacc.Bacc(None, target_bir_lowering=False)

    # pre-chunked host layouts: partition dim first so each input needs
    # only a few large DMAs (the transfer stage is serial; ~900ns fixed
    # semaphore-propagation cost per transfer)
    xT_d = nc.dram_tensor("xT", [128, KC, S], BF16, kind="ExternalInput")
    # W flat layout [k01 (6*128) | k22 (6*128) | v (6*192)], kc-major inside
    # each block, so every DMA slice is >=512B-contiguous per partition
    # (contiguous runs below 512B pay a 2x DMA latency multiplier).
    w_d = nc.dram_tensor("W", [128, KC * WCOLS], BF16, kind="ExternalInput")
    # qT slot 0 = [q_h0 ; q_h1], slot 1 = [q_h2 ; q_h2]
    qT_d = nc.dram_tensor("qT", [128, 2, S], BF16, kind="ExternalInput")
    # ct = [bk(2) | bv(192) | kb(16) | s1(16) | s2(16)]
    ct_d = nc.dram_tensor("ct", [128, 242], F32, kind="ExternalInput")
    dm_d = nc.dram_tensor("dm", [128, 128], BF16, kind="ExternalInput")
    out_d = nc.dram_tensor("outT", [G, 2, 2, 128, 4 * (D + 1)], F32,
                           kind="ExternalOutput")

    with tile.TileContext(nc) as tc:
        with (
            tc.tile_pool(name="const", bufs=1) as cpool,
            tc.tile_pool(name="pt", bufs=4) as ptpool,
            tc.tile_pool(name="ovec", bufs=4) as opool,
            tc.tile_pool(name="psS", bufs=3, space="PSUM") as pss,
            tc.tile_pool(name="psV", bufs=2, space="PSUM") as psv,
        ):
            xT_sb = cpool.tile([128, KC, S], BF16)
            w_sb = cpool.tile([128, KC * WCOLS], BF16)
            qT_sb = cpool.tile([128, 2, S], BF16)
            kTa_sb = cpool.tile([128, S], BF16)    # [k_h0 ; k_h1]
            kTc_sb = cpool.tile([64, S], BF16)     # k_h2
            v_sb = cpool.tile([128, NT, G, D + 1], BF16)
            ct_sb = cpool.tile([128, 242], F32)
            dm_sb = cpool.tile([128, 128], BF16)
            bk_sb = ct_sb[:, 0:2]
            bv_sb = ct_sb[:, 2:194]
            kb_sb = ct_sb[:, 194:210]
            s1_sb = ct_sb[:, 210:226]
            s2_sb = ct_sb[:, 226:242]

            # PE warm-up: throwaway matmuls so the tensor engine's DVFS
            # ramp (slow p-states for the first ~3us of activity) is spent
            # before the first real projection chain arrives.
            wu_sb = cpool.tile([128, 512], BF16)
            wups = pss.tile([128, 1024], F32, tag="ps", name="wups")
            nc.gpsimd.memset(wu_sb, 0.0)
            for i in range(8):
                nc.tensor.matmul(
                    wups[:, 0:512], wu_sb[:, 0:128], wu_sb,
                    start=True, stop=True,
                )

            # --- input DMAs.  The transfer stage is one serial resource, so
            # order = first-use order.  The first two ride the ACT ring: its
            # HWDGE pipeline primes in parallel with the SP ring's, so the
            # first transfer starts ~0.5us earlier, and the ACT sequencer is
            # idle until the first bias-add anyway.  ct (the k bias) must land
            # before the first chain's bias-add. ---
            nc.scalar.dma_start(out=w_sb[:, 0:768], in_=w_d[:, 0:768])
            nc.scalar.dma_start(out=ct_sb, in_=ct_d[:, :])
            nc.scalar.dma_start(out=dm_sb, in_=dm_d[:, :])
            nc.sync.dma_start(out=xT_sb[:, :, 0:256], in_=xT_d[:, :, 0:256])
            nc.sync.dma_start(out=xT_sb[:, :, 256:512],
                              in_=xT_d[:, :, 256:512])
            nc.sync.dma_start(out=qT_sb[:, 0, 0:512], in_=qT_d[:, 0, 0:512])
            nc.sync.dma_start(out=w_sb[:, 768:KC * WCOLS],
                              in_=w_d[:, 768:KC * WCOLS])
            nc.sync.dma_start(out=xT_sb[:, :, 512:1024],
                              in_=xT_d[:, :, 512:1024])
            nc.sync.dma_start(out=xT_sb[:, :, 1024:1536],
                              in_=xT_d[:, :, 1024:1536])
            nc.sync.dma_start(out=xT_sb[:, :, 1536:2048],
                              in_=xT_d[:, :, 1536:2048])
            nc.sync.dma_start(out=qT_sb[:, 0, 512:2048],
                              in_=qT_d[:, 0, 512:2048])
            nc.sync.dma_start(out=qT_sb[0:64, 1, :], in_=qT_d[0:64, 1, :])

            for rep in range(reps):
                # ---- kv projection ----
                def proj_k_chain(ct, c0, w=512, eng=None, early=False):
                    # during block 0 the pv banks are idle (no chains yet)
                    # and a 256-col f32 chain fits a [128,4,65] pv slot, so
                    # early projection work keeps out of the score ps ring
                    if early:
                        ps = psv.tile([128, w], F32, tag="pv",
                                      name=f"kf_{ct}_{c0}")
                    else:
                        ps = pss.tile([128, 1024], F32, tag="ps")
                    # ct=1 (h2) is a single 64-col block: both of pair 1's
                    # score matmuls read it (and q_h2) at partitions 0:64
                    # with tile_position (0,0) — packing the PE quadrants
                    # buys nothing (matmuls are serial), and the walrus
                    # "Fmap and Weight same partition start" rule is kept.
                    rows = 128 if ct == 0 else 64
                    for kc in range(KC):
                        nc.tensor.matmul(
                            ps[0:rows, 0:w],
                            w_sb[:, ct * 768 + kc * rows:
                                 ct * 768 + (kc + 1) * rows],
                            xT_sb[:, kc, c0:c0 + w],
                            start=(kc == 0),
                            stop=(kc == KC - 1),
                        )
                    dst = kTa_sb if ct == 0 else kTc_sb
                    # bias-add engine is chosen per-granule to dodge the
                    # in-order queue behind whichever engine is busy with exps
                    if eng is None or eng is nc.scalar:
                        nc.scalar.add(dst[0:rows, c0:c0 + w],
                                      ps[0:rows, 0:w],
                                      bk_sb[0:rows, ct:ct + 1])
                    else:
                        eng.tensor_scalar_add(dst[0:rows, c0:c0 + w],
                                              ps[0:rows, 0:w],
                                              bk_sb[0:rows, ct:ct + 1])

                def proj_v_tile(t, early=False):
                    if early:
                        ps = psv.tile([128, 192], F32, tag="pv",
                                      name=f"vf_{t}")
                    else:
                        ps = pss.tile([128, 1024], F32, tag="ps")
                    for kc in range(KC):
                        nc.tensor.matmul(
                            ps[:, 0:192],
                            xT_sb[:, kc, t * 128:(t + 1) * 128],
                            w_sb[:, 1152 + kc * 192:1152 + (kc + 1) * 192],
                            start=(kc == 0),
                            stop=(kc == KC - 1),
                        )
                    nc.vector.tensor_add(
                        v_sb[:, t, :, 0:D],
                        ps[:, 0:G * D].rearrange("p (h d) -> p h d", h=G),
                        bv_sb.rearrange("p (h d) -> p h d", h=G),
                    )
                    nc.vector.memset(v_sb[:, t, :, D:D + 1], 1.0)

                # ---- attention ----
                # blocks: (pair, qc).  pair 0 = heads h0/h1, query chunk
                # qc*512; pair 1 = h2 self-paired, chunks 2qc / 2qc+1 on the
                # two psum halves.
                blocks = [(0, qc) for qc in range(4)] + [(1, j) for j in (0, 1)]
                pt_tiles = {}

                def emit_scores(pair, qc, t):
                    ps = pss.tile([128, 1024], F32, tag="ps")
                    kT = kTa_sb if pair == 0 else kTc_sb
                    if pair == 0:
                        qA = qT_sb[0:64, 0, qc * 512:(qc + 1) * 512]
                        qB = qT_sb[64:128, 0, qc * 512:(qc + 1) * 512]
                        kB, posB = kT[64:128], (64, 0)
                    else:
                        qA = qT_sb[0:64, 1, qc * 1024:qc * 1024 + 512]
                        qB = qT_sb[0:64, 1, qc * 1024 + 512:(qc + 1) * 1024]
                        kB, posB = kT[0:64], (0, 0)
                    nc.tensor.matmul(
                        ps[:, 0:512], kT[0:64, t * 128:(t + 1) * 128], qA,
                        start=True, stop=True, tile_position=(0, 0),
                    )
                    nc.tensor.matmul(
                        ps[:, 512:1024], kB[:, t * 128:(t + 1) * 128], qB,
                        start=True, stop=True, tile_position=posB,
                    )
                    return ps

                def emit_exp(pair, qc, t, ps, extra=False, bi=None):
                    pt = ptpool.tile([128, 1024], BF16, tag=f"pt{t}")
                    dve = t in OFFLOAD_T or extra
                    # same-engine mask for DVE tiles avoids a Pool hop (two
                    # extra cross-engine semaphore delays) on the pt path
                    meng = nc.vector if dve else nc.gpsimd
                    if dve:
                        nc.vector.tensor_scalar(
                            pt.bitcast(I16), ps,
                            s1_sb[:, t:t + 1], s2_sb[:, t:t + 1],
                            AluMult, AluAdd,
                        )
                    else:
                        nc.scalar.activation(
                            pt, ps, Exp, bias=kb_sb[:, t:t + 1], scale=SCALE
                        )
                    # zero the q == key block diagonal (on the otherwise-idle
                    # GPSIMD engine; pt lives in SBUF which Pool can access)
                    c = (t % 4) * 128
                    if pair == 0:
                        if t // 4 == qc:
                            meng.tensor_mul(
                                pt[:, c:c + 128], pt[:, c:c + 128], dm_sb)
                            meng.tensor_mul(
                                pt[:, 512 + c:512 + c + 128],
                                pt[:, 512 + c:512 + c + 128], dm_sb)
                    else:
                        if t // 4 == 2 * qc:
                            meng.tensor_mul(
                                pt[:, c:c + 128], pt[:, c:c + 128], dm_sb)
                        elif t // 4 == 2 * qc + 1:
                            meng.tensor_mul(
                                pt[:, 512 + c:512 + c + 128],
                                pt[:, 512 + c:512 + c + 128], dm_sb)
                    pt_tiles[(pair, qc, t)] = pt

                # pv chains: chain (h, qt) covers queries qt*128..+128 of
                # head h.  Four consecutive chains of one head share a
                # 1-bank PSUM tile and one output DMA.
                pv_cur = [None]
                ov_cur = [None, None]

                def pt_col(h, qt, t):
                    if h < 2:
                        key = (0, qt // 4, t)
                        col = 512 * h + (qt % 4) * 128
                    else:
                        key = (1, qt // 8, t)
                        col = 512 * ((qt % 8) // 4) + (qt % 4) * 128
                    return pt_tiles[key][:, col:col + 128]

                def emit_pv_chain(h, qt, i0=0, i1=NT, pv=None, order=None,
                                  ps_pool=False, drain=True):
                    if pv is None:
                        if ps_pool:
                            # tail chains ride the score-psum ring: each new
                            # request lands on the slot its gating exp (or
                            # predecessor's copy) is about to release
                            pv = pss.tile([128, 4, D + 1], F32, tag="ps",
                                          name=f"tl_{rep}_{qt}")
                        elif qt % 4 == 0 and i0 == 0:
                            pv_cur[0] = psv.tile(
                                [128, 4, D + 1], F32, tag="pv",
                                name=f"pv_{rep}_{h}_{qt}")
                        pv = pv if ps_pool else pv_cur[0]
                    for idx in range(i0, i1):
                        t = order[idx] if order else idx
                        nc.tensor.matmul(
                            pv[:, qt % 4, :],
                            pt_col(h, qt, t),
                            v_sb[:, t, h, :],
                            start=(idx == 0),
                            stop=(idx == NT - 1),
                        )
                    if i1 < NT:
                        return pv
                    if drain and qt % 4 == 3:
                        ov = opool.tile([128, 4, D + 1], F32, tag="ov",
                                        name=f"ov_{rep}_{h}_{qt}")
                        nc.vector.tensor_copy(ov, pv)
                        nc.sync.dma_start(
                            out=out_d[h, qt // 8, (qt // 4) % 2, :, :],
                            in_=ov.rearrange("p a b -> p (a b)"),
                        )
                    return pv

                # chains of block bi, in emission order (groups of 4)
                def block_chains(bi):
                    pair, qc = blocks[bi]
                    if pair == 0:
                        return ([(0, 4 * qc + i) for i in range(4)]
                                + [(1, 4 * qc + i) for i in range(4)])
                    return [(2, 8 * qc + i) for i in range(8)]

                # Remaining projection work rides inside the attention step
                # stream, timed to the xT column-slice DMA arrivals.  bwork
                # values are LISTS of thunks (all emitted after that step's
                # score matmuls, before the next step's).
                def vt(t, early=False):
                    return lambda: proj_v_tile(t, early)

                def kch(ct, c0, w=256, eng=None, early=False):
                    return lambda: proj_k_chain(ct, c0, w, eng, early)

                V, A = nc.vector, nc.scalar
                bwork = {
                    0: {
                        0: [kch(0, 256, early=True)],
                        1: [vt(0, True)],
                        2: [vt(1, True), kch(0, 512, eng=V, early=True)],
                        3: [kch(0, 768, eng=A, early=True)],
                        5: [vt(2, True)],
                        7: [kch(0, 1024, eng=V, early=True),
                            kch(0, 1280, eng=A, early=True), vt(3, True)],
                        9: [kch(0, 1536, eng=V, early=True),
                            kch(0, 1792, eng=A, early=True)],
                        11: [vt(4, True), vt(5, True)],
                        13: [vt(6, True), vt(7, True)],
                        15: [vt(8, True), vt(9, True)],
                    },
                    1: {0: [vt(10), vt(11)], 1: [vt(12), vt(13)],
                        2: [vt(14), vt(15)]},
                    2: {st: [kch(1, 512 * n, eng=V), kch(1, 512 * n + 256,
                                                        eng=A)]
                        for n, st in enumerate((0, 2, 4, 6))},
                }

                # first granules 128 wide: score t0/t1 start earliest
                proj_k_chain(0, 0, 128, early=True)
                proj_k_chain(0, 128, 128, early=True)

                # flat software pipeline over all (block, t) steps: scores
                # run LOOKAHEAD steps ahead of exp, across block boundaries
                forder = list(range(8, NT)) + list(range(8))
                steps = [(bi, t) for bi in range(len(blocks))
                         for t in range(NT)]
                n_steps = len(steps)
                final = len(blocks) - 1
                prev_ps = {}
                pvqs = {bi: block_chains(bi - 1)
                        for bi in range(1, len(blocks))}
                tl = {}
                ovt = [None, None]
                fpv = {}
                def emit_tail(pj_pos):
                    # the remaining 6 chains ride score-psum slots in ring
                    # order: slot(pos13) -> ch9, slot(pos14) -> ch10,
                    # slot(pos15) -> ch11, then ch13/14/15 reuse them as the
                    # copies drain.  PE fills the final-exp latency with
                    # presteps instead of idling.
                    C = lambda qt, lo, hi, **kw: emit_pv_chain(
                        2, qt, lo, hi, order=forder, ps_pool=True,
                        drain=False, **kw)

                    def copy(qt, pv, eng):
                        g = (qt - 8) // 4
                        ov = ovt[g]
                        s = qt % 4
                        eng_map = {0: nc.vector.tensor_copy,
                                   1: nc.scalar.copy}
                        eng_map[eng](ov[:, s:s + 1, :], pv[:, s:s + 1, :])

                    def dma(qt):
                        # group 0 ships whole; group 1 ships as two pairs so
                        # the kernel's very last transfer is small
                        g = (qt - 8) // 4
                        if qt == 11:
                            nc.sync.dma_start(
                                out=out_d[2, 1, 0, :, :],
                                in_=ovt[0].rearrange("p a b -> p (a b)"),
                            )
                        elif qt in (13, 15):
                            s = (qt % 4) - 1
                            nc.sync.dma_start(
                                out=out_d[2, 1, 1, :,
                                          s * (D + 1):(s + 2) * (D + 1)],
                                in_=ovt[1][:, s:s + 2, :].rearrange(
                                    "p a b -> p (a b)"),
                            )

                    if pj_pos == 13:
                        ovt[0] = opool.tile([128, 4, D + 1], F32, tag="ov",
                                            name=f"ovt0_{rep}")
                        ovt[1] = opool.tile([128, 4, D + 1], F32, tag="ov",
                                            name=f"ovt1_{rep}")
                        tl[9] = C(9, 0, 14)
                        fpv[8] = emit_pv_chain(2, 8, 13, 14, pv=fpv[8],
                                               order=forder)
                        fpv[12] = emit_pv_chain(2, 12, 13, 14, pv=fpv[12],
                                                order=forder)
                    elif pj_pos == 14:
                        tl[10] = C(10, 0, 15)
                        tl[9] = C(9, 14, 15, pv=tl[9])
                        fpv[8] = emit_pv_chain(2, 8, 14, 15, pv=fpv[8],
                                               order=forder)
                        fpv[12] = emit_pv_chain(2, 12, 14, 15, pv=fpv[12],
                                                order=forder)
                    else:
                        emit_pv_chain(2, 8, 15, 16, pv=fpv[8], order=forder)
                        C(9, 15, 16, pv=tl[9])
                        copy(8, fpv[8], 0)
                        copy(9, tl[9], 1)
                        dma(9)
                        C(10, 15, 16, pv=tl[10])
                        tl[11] = C(11, 0, 16)
                        copy(10, tl[10], 0)
                        copy(11, tl[11], 1)
                        dma(11)
                        emit_pv_chain(2, 12, 15, 16, pv=fpv[12], order=forder)
                        copy(12, fpv[12], 0)
                        tl[13] = C(13, 0, 16)
                        copy(13, tl[13], 1)
                        dma(13)
                        tl[14] = C(14, 0, 16)
                        copy(14, tl[14], 1)
                        tl[15] = C(15, 0, 16)
                        copy(15, tl[15], 0)
                        dma(15)

                for i in range(n_steps + LOOKAHEAD):
                    if i < n_steps:
                        bi, pos = steps[i]
                        t = forder[pos] if bi == final else pos
                        pair, qc = blocks[bi]
                        prev_ps[(bi, t)] = emit_scores(pair, qc, t)
                        if pos == 0 and bi >= 2:
                            # leftover pv chains of earlier blocks
                            for b in range(1, bi):
                                while pvqs.get(b):
                                    emit_pv_chain(*pvqs[b].pop(0))
                        if bi in bwork and pos in bwork[bi]:
                            for thunk in bwork[bi][pos]:
                                thunk()
                        elif pos % 2 == 1 and pos >= 3:
                            # pv chains of the previous block on odd steps
                            # (cross-block lookahead exps land first; block
                            # 0's chains also wait for the last v-tile)
                            if pvqs.get(bi):
                                emit_pv_chain(*pvqs[bi].pop(0))
                        if bi == final and pos == 14:
                            while pvqs[final]:
                                emit_pv_chain(*pvqs[final].pop(0))
                        if bi == final and pos == 15:
                            # chains 8/12 prestep in the two pv banks
                            # (one open accumulation group per bank)
                            fpv = {8: emit_pv_chain(2, 8, 0, 13, order=forder),
                                   12: emit_pv_chain(2, 12, 0, 13,
                                                     order=forder)}
                    j = i - LOOKAHEAD
                    if j >= 0:
                        bj, pj_pos = steps[j]
                        tj = forder[pj_pos] if bj == final else pj_pos
                        pj, qj = blocks[bj]
                        emit_exp(pj, qj, tj, prev_ps.pop((bj, tj)))
                        if bj == final and pj_pos >= 13:
                            emit_tail(pj_pos)


    nc.finalize()
    return nc


_NC = None


def _get_nc():
    global _NC
    if _NC is None:
        _NC = _build_nc()
    return _NC


def _host_prep(hidden_states, embx, expanded_embx, Wkv_w, Wkv_b,
               attention_mask, mlm_mask):
    hs = np.asarray(hidden_states, np.float32)
    qx = np.asarray(expanded_embx, np.float32)
    w = np.asarray(Wkv_w, np.float32)
    bb = np.asarray(Wkv_b, np.float32)
    am = np.asarray(attention_mask).astype(bool)
    mm = np.asarray(mlm_mask).astype(bool)

    valid = (am & ~mm).astype(np.float32)              # (B, S)

    dm = np.ones((128, 128), ml_dtypes.bfloat16)
    np.fill_diagonal(dm, 0.0)

    # per-batch tensors; xT pre-chunked as [128, KC, S]
    xT = [np.ascontiguousarray(
              hs[b].T.astype(ml_dtypes.bfloat16)
              .reshape(KC, 128, S).transpose(1, 0, 2))
          for b in range(B)]
    kbf, s1f, s2f = [], [], []
    for b in range(B):
        v = valid[b]                                   # (S,)
        kb = np.where(v > 0, 0.0, NEG).astype(np.float32)
        s1 = (v * (SA16 * SCALE)).astype(np.float32)
        s2 = (v * SB16).astype(np.float32)
        kbf.append(kb.reshape(NT, 128).T)
        s1f.append(s1.reshape(NT, 128).T)
        s2f.append(s2.reshape(NT, 128).T)

    # per-group weight layouts, flat [128, KC*WCOLS]: blocks
    # [k01 | k22 | v], kc-major inside each block (>=512B contiguous runs)
    wg_l, bk_l, bv_l = [], [], []
    for g in range(4):
        k_cols = slice(192 * g, 192 * g + 192)
        v_cols = slice(768 + 192 * g, 768 + 192 * g + 192)
        wk = w[:, k_cols]                              # (768, 192)
        blocks = [wk[:, 0:128],                        # [k_h0 | k_h1]
                  wk[:, 128:192],                      # k_h2 (once)
                  w[:, v_cols]]                        # v (192)
        flat = np.concatenate(
            [b.reshape(KC, 128, -1).transpose(1, 0, 2).reshape(128, -1)
             for b in blocks], axis=1).astype(ml_dtypes.bfloat16)
        wg_l.append(np.ascontiguousarray(flat))
        bkk = bb[k_cols]
        bk = np.stack([bkk[0:128],
                       np.concatenate([bkk[128:192], bkk[128:192]])], axis=1)
        bk_l.append(bk.astype(np.float32))
        bv_l.append(np.broadcast_to(
            bb[v_cols], (128, G * D)).astype(np.float32))

    in_maps = []
    for c in range(NCORE):
        b, g = divmod(c, 4)
        ct = np.concatenate(
            [bk_l[g], bv_l[g], kbf[b], s1f[b], s2f[b]], axis=1)
        qg = qx[b][:, 192 * g:192 * g + 192].T         # (192, S)
        qt = np.empty((128, 2, S), ml_dtypes.bfloat16)
        qt[0:64, 0, :] = qg[0:64].astype(ml_dtypes.bfloat16)
        qt[64:128, 0, :] = qg[64:128].astype(ml_dtypes.bfloat16)
        qt[0:64, 1, :] = qg[128:192].astype(ml_dtypes.bfloat16)
        qt[64:128, 1, :] = qt[0:64, 1, :]
        in_maps.append(dict(
            xT=xT[b], W=wg_l[g], qT=np.ascontiguousarray(qt),
            ct=np.ascontiguousarray(ct), dm=dm,
        ))
    return in_maps


def _host_post(results, embx, expanded_embx, Wkv_w, Wkv_b):
    ex = np.asarray(embx, np.float32)                  # (B, 1, HID)
    qx = np.asarray(expanded_embx, np.float32)
    w = np.asarray(Wkv_w, np.float32)
    bb = np.asarray(Wkv_b, np.float32)

    # embx key: k/v projections + per-query weights, on host
    kv_eb = ex[:, 0, :] @ w + bb                       # (B, 2*HID)
    k_eb = kv_eb[:, :HID].reshape(B, H, D)
    v_eb = kv_eb[:, HID:].reshape(B, H, D)
    q3 = qx.reshape(B, S, H, D)
    s_eb = np.einsum("bshd,bhd->bsh", q3, k_eb)        # (B, S, H)
    p_eb = np.exp(SCALE * s_eb.astype(np.float64)).astype(np.float32)

    out = np.empty((B, S, HID), np.float32)
    for c in range(NCORE):
        b, g = divmod(c, 4)
        # [G, 2, 2, 128, 4, 65] -> (h, half, group, slot, row) -> (G, S, 65)
        ot = (results[c]["outT"]
              .reshape(G, 2, 2, 128, 4, D + 1)
              .transpose(0, 1, 2, 4, 3, 5)
              .reshape(G, S, D + 1))
        for h in range(G):
            hh = 3 * g + h
            num = ot[h, :, :D] + p_eb[b, :, hh:hh + 1] * v_eb[b, hh]
            den = ot[h, :, D] + p_eb[b, :, hh]
            out[b, :, 192 * g + 64 * h:192 * g + 64 * h + 64] = (
                num / den[:, None]
            )
    return out


def kernel(hidden_states, embx, expanded_embx, Wkv_w, Wkv_b,
           attention_mask, mlm_mask):
    in_maps = _host_prep(hidden_states, embx, expanded_embx, Wkv_w, Wkv_b,
                         attention_mask, mlm_mask)
    nc = _get_nc()
    res = run_bass_kernel_spmd(nc, in_maps, list(range(NCORE)))
    return _host_post(res.results, embx, expanded_embx, Wkv_w, Wkv_b)

